# revision 1
# baseline (speedup 1.0000x reference)
"""Trainium2 Bass kernel for nn_EntityResolutionProcessor.

Strategy: data-parallel over mentions (M=1024 -> 128/core on 8 cores).
On-device per core:
  phase0: blocked cumsum of text -> csum scratch in DRAM (f32); indirect-DMA
          gather of 4 csum rows per mention; mention/context means (f32);
          weights + candidates converted to bf16 DRAM scratch.
  per-mention: feature-major projections (relik-W1a, q/k/v, uni-U1a), s_aa.
  8 macro-tiles of 512 pairs: candidate DMA-transpose, q/k/v projections,
          relik/unirel heads, 2-token attention via sigmoid softmax,
          wo + LN1, FFN, LN2+cosine fused via sufficient statistics.
Activations feature-major [feat->6x128 partitions, pairs]. Matmul operands
bf16 (fp32 psum accumulate); cumsum/means/layernorm lane math in fp32.
"""

from contextlib import ExitStack

import ml_dtypes
import numpy as np

import concourse.bass as bass
import concourse.mybir as mybir
import concourse.tile as tile
from concourse import bacc, bass_isa, bass_utils
from concourse.bass import IndirectOffsetOnAxis, ds, ts

S, D, M, K, H = 4096, 768, 1024, 32, 8
DH = D // H
CTX = 10
NCORES = 8
P = 128
FC = D // P                     # 6 feature chunks
HFC = 4 * D // P                # 24 ffn hidden chunks
M_LOC = M // NCORES             # 128 mentions per core
PAIRS = M_LOC * K               # 4096 pairs per core
NP = 512                        # pairs per macro tile
G = NP // K                     # 16 mentions per macro tile
NMACRO = PAIRS // NP            # 8
NCH = S // P                    # 32 text chunks
ISQ = 1.0 / float(np.sqrt(np.float32(DH)))
EPS_LN = 1e-5
EPS_COS = 1e-8

F32 = mybir.dt.float32
BF16 = mybir.dt.bfloat16
I32 = mybir.dt.int32
AF = mybir.ActivationFunctionType
ALU = mybir.AluOpType

_NC_CACHE = {}


def _gk(ap):
    """view a [128, NP] AP as [128, G, K]"""
    return ap.rearrange("p (g k) -> p g k", g=G)


def _feat_major(w_ap):
    """[in, out] dram AP -> [128, in//128, out] (partition = in % 128)"""
    return w_ap.rearrange("(i p) o -> p i o", p=P)


def _vec6(v_ap, n=FC):
    """[D] dram AP -> [128, n] per-feature layout"""
    return v_ap.rearrange("(i p) -> p i", p=P)


def _build_nc():
    nc = bacc.Bacc(
        "TRN2", target_bir_lowering=False, debug=False, num_devices=NCORES
    )

    def inp(name, shape, dtype=F32):
        return nc.dram_tensor(name, list(shape), dtype, kind="ExternalInput").ap()

    t = {}
    t["txt"] = inp("txt", [S, D])
    t["cand"] = inp("cand", [PAIRS, D])
    t["idx"] = inp("idx", [P, 4], I32)
    t["invl"] = inp("invl", [P, 2])
    t["seltab"] = inp("seltab", [NCH, 2, P])
    t["tri"] = inp("tri", [P, P])          # upper-tri incl (lhsT of L)
    t["tri32"] = inp("tri32", [NCH, NCH])  # strict upper (lhsT of strict L)
    t["ident"] = inp("ident", [P, P])
    t["identb"] = inp("identb", [P, P], BF16)
    t["zrow"] = inp("zrow", [1, D])
    t["hmat"] = inp("hmat", [D, H], BF16)  # head indicator
    t["i8neg"] = inp("i8neg", [H, H], BF16)

    for n, shp in [("relik_w1", [2 * D, D]), ("relik_b1", [D]),
                   ("relik_w2", [D, 1]), ("relik_b2", [1, 1]),
                   ("wq", [D, D]), ("bq", [D]), ("wk", [D, D]), ("bk", [D]),
                   ("wv", [D, D]), ("bv", [D]), ("wo", [D, D]), ("bo", [D]),
                   ("ln1_g", [D]), ("ln1_b", [D]),
                   ("ffn_w1", [D, 4 * D]), ("ffn_b1", [4 * D]),
                   ("ffn_w2", [4 * D, D]), ("ffn_b2", [D]),
                   ("ln2_g", [D]), ("ln2_b", [D]),
                   ("uni_w1", [2 * D, D]), ("uni_b1", [D]),
                   ("uni_w2", [D, D]), ("uni_b2", [1, D])]:
        t[n] = inp(n, shp)

    t["out"] = nc.dram_tensor("out", [3, PAIRS], F32, kind="ExternalOutput").ap()
    t["csum"] = nc.dram_tensor("csum_scratch", [S + 1, D], F32).ap()
    # bf16 scratch: candidates + streamed weights (strip-major layouts)
    t["cand_bf"] = nc.dram_tensor("cand_bf", [PAIRS, D], BF16).ap()
    for n, noc, nic in [("wq_bf", FC, FC), ("wk_bf", FC, FC),
                        ("wv_bf", FC, FC), ("wo_bf", FC, FC),
                        ("w1b_bf", FC, FC), ("u1b_bf", FC, FC),
                        ("fw1_bf", HFC, FC), ("fw2_bf", FC, HFC)]:
        t[n] = nc.dram_tensor(n, [noc, P, nic * P], BF16).ap()

    with tile.TileContext(nc) as tc:
        _body(nc, tc, t)
    nc.compile()
    return nc


def _body(nc, tc, t):
    with ExitStack() as _ctx:
        _body_inner(nc, tc, t, _ctx)


def _body_inner(nc, tc, t, _ctx):
    mm = lambda *a, **k: nc.tensor.matmul(*a, **k)

    # ---------------- pools ----------------
    psum = _ctx.enter_context(tc.tile_pool(name="psum", bufs=1, space="PSUM"))
    res = _ctx.enter_context(tc.tile_pool(name="res", bufs=1))

    def ps_mm(shape=(P, NP), dtype=F32):
        return psum.tile(list(shape), dtype, tag="mm", bufs=3,
                         padded_shape=[P, NP], name="ps_mm")

    def ps_score():
        return psum.tile([8, NP], F32, tag="score", bufs=1, name="ps_score")

    def ps_stat():
        # stats tile: MM groups land at base partitions 0 and 32
        return psum.tile([P, NP], F32, tag="stat", bufs=2, name="ps_stat")

    def ps_head():
        return psum.tile([1, NP], F32, tag="head", bufs=2, name="ps_head")

    # ---------------- resident constants ----------------
    def load_res(name, ap_src, shape, dtype=F32, conv=False):
        tl = res.tile(list(shape), dtype, name=name)
        nc.gpsimd.dma_start(tl[:], ap_src)
        return tl

    tri_sb = load_res("tri_sb", t["tri"][:], [P, P])
    tri32_sb = load_res("tri32_sb", t["tri32"][:], [NCH, NCH])
    ident_sb = load_res("ident_sb", t["ident"][:], [P, P])
    identb_sb = load_res("identb_sb", t["identb"][:], [P, P], BF16)
    i8neg_sb = load_res("i8neg_sb", t["i8neg"][:], [H, H], BF16)
    # H in two layouts: lhsT for head-reduce [128,6,8]; lhsT for bcast [8,6,128]
    h_sb = load_res("h_sb", t["hmat"].rearrange("(c p) h -> p c h", p=P),
                    [P, FC, H], BF16)
    ht_sb = load_res("ht_sb", t["hmat"].rearrange("(c p) h -> h c p", p=P),
                     [H, FC, P], BF16)
    negh_sb = res.tile([P, FC, H], BF16, name="negh_sb")
    nc.vector.tensor_scalar_mul(negh_sb[:], h_sb[:], -1.0)

    idx_sb = load_res("idx_sb", t["idx"][:], [P, 4], I32)
    invl_sb = load_res("invl_sb", t["invl"][:], [P, 2])
    sel_sb = load_res("sel_sb", t["seltab"][:], [NCH, 2, P])

    bq_sb = load_res("bq_sb", _vec6(t["bq"]), [P, FC])
    bk_sb = load_res("bk_sb", _vec6(t["bk"]), [P, FC])
    bv_sb = load_res("bv_sb", _vec6(t["bv"]), [P, FC])
    bo_sb = load_res("bo_sb", _vec6(t["bo"]), [P, FC])
    rb1_sb = load_res("rb1_sb", _vec6(t["relik_b1"]), [P, FC])
    ub1_sb = load_res("ub1_sb", _vec6(t["uni_b1"]), [P, FC])
    fb1_sb = load_res("fb1_sb", _vec6(t["ffn_b1"], HFC), [P, HFC])
    fb2_sb = load_res("fb2_sb", _vec6(t["ffn_b2"]), [P, FC])
    l1g_sb = load_res("l1g_sb", _vec6(t["ln1_g"]), [P, FC])
    l1b_sb = load_res("l1b_sb", _vec6(t["ln1_b"]), [P, FC])
    l2g_sb = load_res("l2g_sb", _vec6(t["ln2_g"]), [P, FC])
    l2b_sb = load_res("l2b_sb", _vec6(t["ln2_b"]), [P, FC])
    rw2_sb = load_res("rw2_sb",
                      t["relik_w2"].rearrange("(c p) o -> p c o", p=P),
                      [P, FC, 1], BF16, conv=True)
    rb2_sb = load_res("rb2_sb", t["relik_b2"][:], [1, 1])

    ones_sb = res.tile([P, 1], BF16, name="ones_sb")
    nc.vector.memset(ones_sb[:], 1.0)
    onesf_sb = res.tile([P, 1], F32, name="onesf_sb")
    nc.vector.memset(onesf_sb[:], 1.0)
    ones_row = res.tile([1, P], BF16, name="ones_row")
    nc.vector.memset(ones_row[:], 1.0)

    # stats lhsT [128, 6, 3]: cols = [1, g2^2, g2*b2] per feature chunk
    sl3_sb = res.tile([P, FC, 3], BF16, name="sl3_sb")
    g2sq_sb = res.tile([P, FC], F32, name="g2sq_sb")
    g2b2_sb = res.tile([P, FC], F32, name="g2b2_sb")
    nc.vector.tensor_mul(g2sq_sb[:], l2g_sb[:], l2g_sb[:])
    nc.vector.tensor_mul(g2b2_sb[:], l2g_sb[:], l2b_sb[:])
    for c in range(FC):
        nc.vector.tensor_copy(sl3_sb[:, c, 0:1], ones_sb[:])
        nc.vector.tensor_copy(sl3_sb[:, c, 1:2], g2sq_sb[:, c:c + 1])
        nc.vector.tensor_copy(sl3_sb[:, c, 2:3], g2b2_sb[:, c:c + 1])

    # scalar reductions of bias/gain vectors (each -> [1,1] on partition 0)
    def vec_sum(name, vecs):
        tmp = res.tile([P, FC], F32, name=name + "_t")
        if len(vecs) == 1:
            nc.vector.tensor_copy(tmp[:], vecs[0][:])
        else:
            nc.vector.tensor_mul(tmp[:], vecs[0][:], vecs[1][:])
            for v in vecs[2:]:
                nc.vector.tensor_mul(tmp[:], tmp[:], v[:])
        red = res.tile([P, 1], F32, name=name + "_r")
        nc.vector.tensor_reduce(red[:], tmp[:], axis=mybir.AxisListType.X,
                                op=ALU.add)
        pR = ps_head()
        mm(pR[:, 0:1], red[:], onesf_sb[:], start=True, stop=True)
        arr = res.tile([1, 1], F32, name=name)
        nc.vector.tensor_copy(arr[:], pR[:, 0:1])
        return arr[0:1, 0:1]

    s_bo = vec_sum("s_bo", [bo_sb])
    s_fb2 = vec_sum("s_fb2", [fb2_sb])
    s_g2 = vec_sum("s_g2", [l2g_sb, l2g_sb])
    s_gb = vec_sum("s_gb", [l2g_sb, l2b_sb])
    s_bb = vec_sum("s_bb", [l2b_sb, l2b_sb])
    s_g2f = vec_sum("s_g2f", [l2g_sb, l2g_sb, fb2_sb])
    s_gbf = vec_sum("s_gbf", [l2g_sb, l2b_sb, fb2_sb])

    u2rs_sb = res.tile([P, FC], BF16, name="u2rs_sb")
    b2m_sb = res.tile([1, 1], F32, name="b2m_sb")

    # per-mention outputs (feature-major [128, 6, 128])
    m_T = res.tile([P, FC, P], F32, name="m_T")     # f32: residual source
    m_Tb = res.tile([P, FC, P], BF16, name="m_Tb")  # bf16: matmul rhs
    c_Tb = res.tile([P, FC, P], BF16, name="c_Tb")
    m_q = res.tile([P, FC, P], BF16, name="m_q")
    m_k = res.tile([P, FC, P], BF16, name="m_k")
    m_v = res.tile([P, FC, P], BF16, name="m_v")
    m_relik = res.tile([P, FC, P], BF16, name="m_relik")
    c_uni = res.tile([P, FC, P], BF16, name="c_uni")
    s_aa_sb = res.tile([H, P], BF16, name="s_aa_sb")

    # ================= phase 0: csum + gather + bf16 conversion ==========
    with tc.tile_pool(name="p0", bufs=1) as p0:
        # uni_w2 row-sums (once)
        u2_sb = p0.tile([P, FC, D], F32, name="u2_sb")
        nc.gpsimd.dma_start(u2_sb[:], _feat_major(t["uni_w2"]))
        u2r_f = p0.tile([P, FC], F32, name="u2r_f")
        nc.vector.tensor_reduce(u2r_f[:], u2_sb[:],
                                axis=mybir.AxisListType.X, op=ALU.add)
        nc.vector.tensor_copy(u2rs_sb[:], u2r_f[:])
        ub2_sb = p0.tile([1, D], F32, name="ub2_sb")
        nc.gpsimd.dma_start(ub2_sb[:], t["uni_b2"][:])
        b2r = p0.tile([1, 1], F32, name="b2r")
        nc.vector.tensor_reduce(b2r[:], ub2_sb[:], axis=mybir.AxisListType.X,
                                op=ALU.add)
        nc.scalar.activation(b2m_sb[:], b2r[:], AF.Copy, scale=1.0 / D)

        # ---- bf16 weight conversion into strip-major scratch ----
        for src_ap, dst, noc, nic in [
            (_feat_major(t["wq"]), t["wq_bf"], FC, FC),
            (_feat_major(t["wk"]), t["wk_bf"], FC, FC),
            (_feat_major(t["wv"]), t["wv_bf"], FC, FC),
            (_feat_major(t["wo"]), t["wo_bf"], FC, FC),
            (_feat_major(t["relik_w1"][D:]), t["w1b_bf"], FC, FC),
            (_feat_major(t["uni_w1"][D:]), t["u1b_bf"], FC, FC),
            (_feat_major(t["ffn_w1"]), t["fw1_bf"], HFC, FC),
            (_feat_major(t["ffn_w2"]), t["fw2_bf"], FC, HFC),
        ]:
            for oc in range(noc):
                nc.gpsimd.dma_start(
                    dst[oc].rearrange("p (i q) -> p i q", q=P),
                    src_ap[:, :, ts(oc, P)])

        # ---- candidates to bf16 (converting DRAM->DRAM DMA) ----
        for c in range(4):
            q = PAIRS // 4
            nc.gpsimd.dma_start(t["cand_bf"][c * q:(c + 1) * q, :],
                                t["cand"][c * q:(c + 1) * q, :])

        # ---- cumsum ----
        totals_sb = p0.tile([NCH, D], F32, name="totals_sb")
        nc.gpsimd.dma_start(t["csum"][0:1, :], t["zrow"][:])

        for c in range(NCH):
            txt_c = p0.tile([P, D], F32, tag="txtc", bufs=3, name="txt_c")
            nc.gpsimd.dma_start(txt_c[:], t["txt"][c * P:(c + 1) * P, :])
            pre_sb = p0.tile([P, D], F32, tag="pre", bufs=3, name="pre_sb")
            for half in range(2):
                sl = ds(half * 384, 384)
                pA = ps_mm((P, 384))
                mm(pA[:], tri_sb[:], txt_c[:, sl], start=True, stop=True)
                nc.any.tensor_copy(pre_sb[:, sl], pA[:])
            nc.gpsimd.dma_start(t["csum"][1 + c * P: 1 + (c + 1) * P, :],
                                pre_sb[:])
            nc.gpsimd.dma_start(totals_sb[c:c + 1, :], pre_sb[P - 1:P, :])

        offs_sb = p0.tile([NCH, D], F32, name="offs_sb")
        for half in range(2):
            sl = ds(half * 384, 384)
            pA = ps_mm((NCH, 384))
            mm(pA[:], tri32_sb[:], totals_sb[:, sl], start=True, stop=True)
            nc.any.tensor_copy(offs_sb[:, sl], pA[:])

        # ---- gathers + means ----
        gath = []
        for j in range(4):
            g_t = p0.tile([P, D], F32, tag=f"g{j}", name=f"g_{j}")
            nc.gpsimd.indirect_dma_start(
                out=g_t[:], out_offset=None, in_=t["csum"][:],
                in_offset=IndirectOffsetOnAxis(ap=idx_sb[:, j:j + 1], axis=0),
            )
            gath.append(g_t)

        def mean_tile(out_name, gp, gm, selcol, inv_col):
            o_t = p0.tile([P, D], F32, name=out_name)
            dif = p0.tile([P, D], F32, tag="dif", bufs=2, name="dif")
            nc.vector.tensor_tensor(dif[:], gath[gp][:], gath[gm][:],
                                    op=ALU.subtract)
            for half in range(2):
                sl = ds(half * 384, 384)
                pA = ps_mm((P, 384))
                mm(pA[:], sel_sb[:, selcol, :], offs_sb[:, sl],
                   start=True, stop=True)
                nc.vector.tensor_tensor(o_t[:, sl], pA[:], dif[:, sl],
                                        op=ALU.add)
            nc.vector.tensor_scalar_mul(o_t[:], o_t[:],
                                        invl_sb[:, inv_col:inv_col + 1])
            return o_t

        mention_rm = mean_tile("mention_rm", 0, 1, 0, 0)
        ctx_rm = mean_tile("ctx_rm", 2, 3, 1, 1)

        for src, dstf, dstb in ((mention_rm, m_T, m_Tb),
                                (ctx_rm, None, c_Tb)):
            for fc in range(FC):
                pT = ps_mm((P, P))
                nc.tensor.transpose(pT[:], src[:, ts(fc, P)], ident_sb[:])
                if dstf is not None:
                    nc.vector.tensor_copy(dstf[:, fc, :], pT[:])
                nc.any.tensor_copy(dstb[:, fc, :], pT[:])

    # ================= pools for the main phase =================
    wts = _ctx.enter_context(tc.tile_pool(name="wts", bufs=1))
    act = _ctx.enter_context(tc.tile_pool(name="act", bufs=1))
    lane = _ctx.enter_context(tc.tile_pool(name="lane", bufs=1))

    def load_strip(bf_dram, oc):
        """stream bf16 weight strip [128, 6, 128] for out-chunk oc"""
        st = wts.tile([P, FC, P], BF16, tag="wstrip", bufs=6, name="w_strip")
        nc.gpsimd.dma_start(st[:],
                          bf_dram[oc].rearrange("p (i q) -> p i q", q=P))
        return st

    def load_strip_conv(w_fm_ap, oc):
        """one-shot converting load (per-mention phase)"""
        st = wts.tile([P, FC, P], BF16, tag="wstrip", bufs=6, name="w_strip")
        nc.gpsimd.dma_start(st[:], w_fm_ap[:, :, ts(oc, P)])
        return st

    def unit(tag, name, bufs=1):
        return act.tile([P, FC, NP], BF16, tag=tag, bufs=bufs, name=name)

    def chunk_t(name):
        return act.tile([P, NP], BF16, tag="tt", bufs=3, name=name)

    # ---------- per-mention projections (bf16, N=128) ----------
    for w_ap, b_sb, out_t, src in (
        (_feat_major(t["wq"]), bq_sb, m_q, m_Tb),
        (_feat_major(t["wk"]), bk_sb, m_k, m_Tb),
        (_feat_major(t["wv"]), bv_sb, m_v, m_Tb),
        (_feat_major(t["relik_w1"][:D]), rb1_sb, m_relik, m_Tb),
        (_feat_major(t["uni_w1"][:D]), ub1_sb, c_uni, c_Tb),
    ):
        for oc in range(FC):
            st = load_strip_conv(w_ap, oc)
            pA = ps_mm((P, P))
            for ic in range(FC):
                mm(pA[:], st[:, ic, :], src[:, ic, :],
                   start=(ic == 0), stop=(ic == FC - 1))
            nc.scalar.activation(out_t[:, oc, :], pA[:], AF.Identity,
                                 bias=b_sb[:, oc:oc + 1])

    # s_aa [8, 128]
    mprod = wts.tile([P, FC, P], BF16, tag="wstrip", bufs=6, name="mprod")
    for c in range(FC):
        nc.vector.tensor_mul(mprod[:, c, :], m_q[:, c, :], m_k[:, c, :])
    pS = ps_score()
    for c in range(FC):
        mm(pS[:, :P], h_sb[:, c, :], mprod[:, c, :],
           start=(c == 0), stop=(c == FC - 1))
    nc.any.tensor_copy(s_aa_sb[:], pS[:, :P])

    # ================= macro-tile loop =================
    for mt in range(NMACRO):
        g0 = mt * G
        gsl = ds(g0, G)

        lane_seq = [0]

        def lane_t(name, parts=1):
            lane_seq[0] += 1
            return lane.tile([parts, NP], F32, tag=name, bufs=1,
                             name=f"{name}_{lane_seq[0]}")

        def mview(mt_tile, c):
            """mention-side bcast view [128, G, K]"""
            return mt_tile[:, c, gsl, None].to_broadcast([P, G, K])

        # ---- candidate load + PE transpose (bf16) ----
        cand_rm = act.tile([P, 4, D], BF16, tag="cand_rm", bufs=1,
                           name="cand_rm")
        nc.gpsimd.dma_start(
            cand_rm[:],
            t["cand_bf"].rearrange("(q p) d -> p q d", p=P)[:, ds(4 * mt, 4), :])
        candT = unit("candT", "candT")
        for fc in range(FC):
            pT = ps_mm(dtype=BF16)
            for pc in range(4):
                nc.tensor.transpose(pT[:, ts(pc, P)],
                                    cand_rm[:, pc, ts(fc, P)], identb_sb[:])
            nc.vector.tensor_copy(candT[:, fc, :], pT[:])

        # ---- k/v projections ----
        k_b = unit("B", "k_b")
        v_b = unit("C", "v_b")
        for wbf, b_sb, out_t in ((t["wk_bf"], bk_sb, k_b),
                                 (t["wv_bf"], bv_sb, v_b)):
            for oc in range(FC):
                st = load_strip(wbf, oc)
                pA = ps_mm()
                for ic in range(FC):
                    mm(pA[:], st[:, ic, :], candT[:, ic, :],
                       start=(ic == 0), stop=(ic == FC - 1))
                nc.scalar.activation(out_t[:, oc, :], pA[:], AF.Identity,
                                     bias=b_sb[:, oc:oc + 1])

        # ---- relik / unirel heads ----
        for wbf, madd, hname, wv2, bias_ap, outrow, fn, scale in (
            (t["w1b_bf"], m_relik, "h_r", rw2_sb, rb2_sb[:], 0,
             AF.Identity, 1.0),
            (t["u1b_bf"], c_uni, "h_u", u2rs_sb, b2m_sb[:], 2,
             AF.Sigmoid, 1.0 / D),
        ):
            h_head = unit("hh", hname, bufs=2)
            for oc in range(FC):
                st = load_strip(wbf, oc)
                pA = ps_mm()
                for ic in range(FC):
                    mm(pA[:], st[:, ic, :], candT[:, ic, :],
                       start=(ic == 0), stop=(ic == FC - 1))
                nc.vector.tensor_tensor(_gk(h_head[:, oc, :]), _gk(pA[:]),
                                        mview(madd, oc), op=ALU.add)
                nc.scalar.activation(h_head[:, oc, :], h_head[:, oc, :],
                                     AF.Relu)
            pH = ps_head()
            for c in range(FC):
                if wv2 is rw2_sb:
                    lhsT = wv2[:, c, :]
                else:
                    lhsT = wv2[:, c:c + 1]
                mm(pH[:], lhsT, h_head[:, c, :],
                   start=(c == 0), stop=(c == FC - 1))
            osl = lane_t("osl_" + hname)
            nc.scalar.activation(osl[:], pH[:], fn, bias=bias_ap, scale=scale)
            nc.gpsimd.dma_start(t["out"][outrow:outrow + 1, ts(mt, NP)], osl[:])

        # ---- attention scores ----
        pAB = ps_score()
        for c in range(FC):
            pr1 = chunk_t("pr1")
            nc.vector.tensor_tensor(_gk(pr1[:]), _gk(k_b[:, c, :]),
                                    mview(m_q, c), op=ALU.mult)
            mm(pAB[:], h_sb[:, c, :], pr1[:], start=(c == 0), stop=False)
        mm(pAB[:], i8neg_sb[:],
           s_aa_sb[:, gsl, None].to_broadcast([H, G, K]),
           start=False, stop=True)
        p_ab = act.tile([H, NP], BF16, tag="p_ab", bufs=2, name="p_ab")
        nc.scalar.activation(p_ab[:], pAB[:], AF.Sigmoid, scale=ISQ)

        pBA = ps_score()
        first = True
        for c in range(FC):
            stq = load_strip(t["wq_bf"], c)
            pQ = ps_mm()
            for ic in range(FC):
                mm(pQ[:], stq[:, ic, :], candT[:, ic, :],
                   start=(ic == 0), stop=(ic == FC - 1))
            q_c = chunk_t("q_c")
            nc.scalar.activation(q_c[:], pQ[:], AF.Identity,
                                 bias=bq_sb[:, c:c + 1])
            pr2 = chunk_t("pr2")
            nc.vector.tensor_tensor(_gk(pr2[:]), _gk(q_c[:]), mview(m_k, c),
                                    op=ALU.mult)
            mm(pBA[:], h_sb[:, c, :], pr2[:], start=first, stop=False)
            first = False
            pr3 = chunk_t("pr3")
            nc.vector.tensor_mul(pr3[:], q_c[:], k_b[:, c, :])
            mm(pBA[:], negh_sb[:, c, :], pr3[:],
               start=False, stop=(c == FC - 1))
        p_ba = act.tile([H, NP], BF16, tag="p_ba", bufs=2, name="p_ba")
        nc.scalar.activation(p_ba[:], pBA[:], AF.Sigmoid, scale=ISQ)

        # ---- attention outputs ----
        o_a = unit("F", "o_a")
        o_b = unit("G", "o_b")
        for c in range(FC):
            dv = chunk_t("dv")
            nc.vector.tensor_tensor(_gk(dv[:]), _gk(v_b[:, c, :]),
                                    mview(m_v, c), op=ALU.subtract)
            pBC = ps_mm()
            mm(pBC[:], ht_sb[:, c, :], p_ab[:], start=True, stop=True)
            nc.vector.tensor_mul(o_a[:, c, :], pBC[:], dv[:])
            nc.vector.tensor_tensor(_gk(o_a[:, c, :]), _gk(o_a[:, c, :]),
                                    mview(m_v, c), op=ALU.add)
            pBC2 = ps_mm()
            mm(pBC2[:], ht_sb[:, c, :], p_ba[:], start=True, stop=True)
            nc.vector.tensor_mul(o_b[:, c, :], pBC2[:], dv[:])
            nc.vector.tensor_tensor(o_b[:, c, :], v_b[:, c, :], o_b[:, c, :],
                                    op=ALU.subtract)

        # ---- wo + residual ----
        r_a = unit("hh", "r_a", bufs=2)
        r_b = unit("hh", "r_b", bufs=2)
        for oc in range(FC):
            st = load_strip(t["wo_bf"], oc)
            pA = ps_mm()
            for ic in range(FC):
                mm(pA[:], st[:, ic, :], o_a[:, ic, :],
                   start=(ic == 0), stop=(ic == FC - 1))
            nc.vector.tensor_tensor(_gk(r_a[:, oc, :]), _gk(pA[:]),
                                    mview(m_T, oc), op=ALU.add)
            pB = ps_mm()
            for ic in range(FC):
                mm(pB[:], st[:, ic, :], o_b[:, ic, :],
                   start=(ic == 0), stop=(ic == FC - 1))
            nc.vector.tensor_tensor(r_b[:, oc, :], pB[:], candT[:, oc, :],
                                    op=ALU.add)

        # ---- LN1 (general gains) -> x1 ----
        def layernorm1(r_t, x1_t, tok):
            pSt = ps_stat()
            for c in range(FC):
                sq = chunk_t("sq")
                nc.scalar.activation(sq[:], r_t[:, c, :], AF.Square,
                                     bias=bo_sb[:, c:c + 1])
                mm(pSt[0:1, :], ones_sb[:], r_t[:, c, :],
                   start=(c == 0), stop=(c == FC - 1))
                mm(pSt[32:33, :], ones_sb[:], sq[:],
                   start=(c == 0), stop=(c == FC - 1))
            mu = lane_t("mu" + tok)
            nc.vector.tensor_scalar(mu[:], pSt[0:1, :], s_bo, 1.0 / D,
                                    op0=ALU.add, op1=ALU.mult)
            var = lane_t("var" + tok)
            nc.vector.tensor_mul(var[:], mu[:], mu[:])
            nc.vector.scalar_tensor_tensor(var[:], pSt[32:33, :], 1.0 / D,
                                           var[:], op0=ALU.mult,
                                           op1=ALU.subtract)
            rstd = lane_t("rstd" + tok)
            nc.vector.tensor_scalar_add(var[:], var[:], EPS_LN)
            nc.scalar.activation(rstd[:], var[:], AF.Sqrt)
            nc.vector.reciprocal(rstd[:], rstd[:])
            mubf = act.tile([1, NP], BF16, tag="mubf", bufs=2, name="mubf")
            rstdbf = act.tile([1, NP], BF16, tag="rstdbf", bufs=2,
                              name="rstdbf")
            nc.vector.tensor_copy(mubf[:], mu[:])
            nc.vector.tensor_copy(rstdbf[:], rstd[:])
            mu_bc = ps_mm()
            rstd_bc = ps_mm()
            mm(mu_bc[:], ones_row[:], mubf[:], start=True, stop=True)
            mm(rstd_bc[:], ones_row[:], rstdbf[:], start=True, stop=True)
            for c in range(FC):
                nc.vector.tensor_tensor(x1_t[:, c, :], r_t[:, c, :],
                                        mu_bc[:], op=ALU.subtract)
                nc.vector.scalar_tensor_tensor(
                    x1_t[:, c, :], x1_t[:, c, :], bo_sb[:, c:c + 1],
                    rstd_bc[:], op0=ALU.add, op1=ALU.mult)
                nc.vector.tensor_scalar(
                    x1_t[:, c, :], x1_t[:, c, :], l1g_sb[:, c:c + 1],
                    l1b_sb[:, c:c + 1], op0=ALU.mult, op1=ALU.add)

        x1_a = unit("A", "x1_a")
        x1_b = unit("B", "x1_b")
        layernorm1(r_a, x1_a, "a")
        layernorm1(r_b, x1_b, "b")

        # ---- FFN (both tokens share each weight strip) ----
        h_a = act.tile([P, HFC, NP], BF16, tag="h", bufs=1, name="h_a")
        # token-b hidden aliases four unit tags that are dead by now
        hb = [unit("candT", "hb0"), unit("G", "hb1"),
              unit("F", "hb2"), unit("hh", "hb3", bufs=2)]

        def ha_c(hc):
            return h_a[:, hc, :]

        def hb_c(hc):
            return hb[hc // FC][:, hc % FC, :]

        for hc in range(HFC):
            st = load_strip(t["fw1_bf"], hc)
            for x1_t, hcs in ((x1_a, ha_c), (x1_b, hb_c)):
                pA = ps_mm()
                for ic in range(FC):
                    mm(pA[:], st[:, ic, :], x1_t[:, ic, :],
                       start=(ic == 0), stop=(ic == FC - 1))
                nc.scalar.activation(hcs(hc), pA[:],
                                     AF.Relu, bias=fb1_sb[:, hc:hc + 1])
        r2_a = unit("C2", "r2_a")
        r2_b = unit("D", "r2_b")
        for oc in range(FC):
            stw = wts.tile([P, HFC, P], BF16, tag="w2strip", bufs=2,
                           name="stw")
            nc.gpsimd.dma_start(
                stw[:],
                t["fw2_bf"][oc].rearrange("p (i q) -> p i q", q=P))
            for x1_t, hcs, r2_t in ((x1_a, ha_c, r2_a), (x1_b, hb_c, r2_b)):
                pA = ps_mm()
                for hc in range(HFC):
                    mm(pA[:], stw[:, hc, :], hcs(hc),
                       start=(hc == 0), stop=(hc == HFC - 1))
                nc.vector.tensor_tensor(r2_t[:, oc, :], pA[:],
                                        x1_t[:, oc, :], op=ALU.add)

        # ---- LN2 + cosine via sufficient statistics ----
        def ln2_stats(r2_t, tok):
            pSt = ps_stat()
            for c in range(FC):
                sq = chunk_t("sq")
                nc.scalar.activation(sq[:], r2_t[:, c, :], AF.Square,
                                     bias=fb2_sb[:, c:c + 1])
                mm(pSt[0:1, :], sl3_sb[:, c, 0:1], r2_t[:, c, :],
                   start=(c == 0), stop=(c == FC - 1))
                mm(pSt[32:33, :], sl3_sb[:, c, 1:2], r2_t[:, c, :],
                   start=(c == 0), stop=(c == FC - 1))
                mm(pSt[64:65, :], sl3_sb[:, c, 2:3], r2_t[:, c, :],
                   start=(c == 0), stop=(c == FC - 1))
                mm(pSt[96:97, :], sl3_sb[:, c, 0:1], sq[:],
                   start=(c == 0), stop=(c == FC - 1),
                   tile_position=(0, 96))
            pS2 = ps_stat()
            for c in range(FC):
                sq2 = chunk_t("sq2")
                nc.scalar.activation(sq2[:], r2_t[:, c, :], AF.Square,
                                     bias=fb2_sb[:, c:c + 1])
                mm(pS2[0:1, :], sl3_sb[:, c, 1:2], sq2[:],
                   start=(c == 0), stop=(c == FC - 1))
            # evict the five stats rows into base-0 lane tiles, folding the
            # constant fb2 corrections
            sz = lane_t("sz" + tok)
            nc.vector.tensor_scalar_add(sz[:], pSt[0:1, :], s_fb2)
            g2z = lane_t("g2z" + tok)
            nc.vector.tensor_scalar_add(g2z[:], pSt[32:33, :], s_g2f)
            gbz = lane_t("gbz" + tok)
            nc.vector.tensor_scalar_add(gbz[:], pSt[64:65, :], s_gbf)
            sq_s = lane_t("sq" + tok)
            nc.vector.tensor_copy(sq_s[:], pSt[96:97, :])
            g2q = lane_t("g2q" + tok)
            nc.vector.tensor_copy(g2q[:], pS2[0:1, :])
            return sz, g2z, gbz, sq_s, g2q

        stats_a = ln2_stats(r2_a, "a")
        stats_b = ln2_stats(r2_b, "b")
        pX = ps_head()
        for c in range(FC):
            rr = chunk_t("rr")
            nc.vector.tensor_scalar_add(rr[:], r2_b[:, c, :],
                                        fb2_sb[:, c:c + 1])
            nc.vector.scalar_tensor_tensor(rr[:], r2_a[:, c, :],
                                           fb2_sb[:, c:c + 1], rr[:],
                                           op0=ALU.add, op1=ALU.mult)
            mm(pX[:], sl3_sb[:, c, 1:2], rr[:],
               start=(c == 0), stop=(c == FC - 1))

        # lane algebra for cosine
        def ln2_lane(stats, tok):
            sz, g2z, gbz, sq_s, g2q = stats
            muz = lane_t("muz" + tok)
            nc.vector.tensor_scalar_mul(muz[:], sz[:], 1.0 / D)
            var = lane_t("var2" + tok)
            nc.vector.tensor_mul(var[:], muz[:], muz[:])
            nc.vector.scalar_tensor_tensor(var[:], sq_s[:], 1.0 / D,
                                           var[:], op0=ALU.mult,
                                           op1=ALU.subtract)
            rstd = lane_t("rstd2" + tok)
            nc.vector.tensor_scalar_add(var[:], var[:], EPS_LN)
            nc.scalar.activation(rstd[:], var[:], AF.Sqrt)
            nc.vector.reciprocal(rstd[:], rstd[:])
            return muz, rstd, g2z, gbz, g2q

        mua, rsta, g2za, gbza, g2qa = ln2_lane(stats_a, "a")
        mub2, rstb, g2zb, gbzb, g2qb = ln2_lane(stats_b, "b")

        def gbt(mu, rstd, gbz, name):
            o_t = lane_t(name)
            nc.vector.tensor_scalar_mul(o_t[:], mu[:], s_gb)
            nc.vector.tensor_tensor(o_t[:], gbz[:], o_t[:], op=ALU.subtract)
            nc.vector.tensor_mul(o_t[:], o_t[:], rstd[:])
            return o_t

        gbta = gbt(mua, rsta, gbza, "gbta")
        gbtb = gbt(mub2, rstb, gbzb, "gbtb")

        def normsq(mu, rstd, g2z, g2q, gbt_t, name):
            o_t = lane_t(name)
            nc.vector.tensor_scalar_mul(o_t[:], mu[:], s_g2)
            nc.vector.scalar_tensor_tensor(o_t[:], g2z[:], -2.0, o_t[:],
                                           op0=ALU.mult, op1=ALU.add)
            nc.vector.tensor_mul(o_t[:], o_t[:], mu[:])
            nc.vector.tensor_add(o_t[:], o_t[:], g2q[:])
            nc.vector.tensor_mul(o_t[:], o_t[:], rstd[:])
            nc.vector.tensor_mul(o_t[:], o_t[:], rstd[:])
            nc.vector.scalar_tensor_tensor(o_t[:], gbt_t[:], 2.0, o_t[:],
                                           op0=ALU.mult, op1=ALU.add)
            nc.vector.tensor_scalar_add(o_t[:], o_t[:], s_bb)
            return o_t

        n2a = normsq(mua, rsta, g2za, g2qa, gbta, "n2a")
        n2b = normsq(mub2, rstb, g2zb, g2qb, gbtb, "n2b")

        d01 = lane_t("d01")
        nc.vector.tensor_scalar_mul(d01[:], mub2[:], s_g2)
        nc.vector.tensor_tensor(d01[:], d01[:], g2zb[:], op=ALU.subtract)
        nc.vector.tensor_mul(d01[:], d01[:], mua[:])
        t2 = lane_t("t2")
        nc.vector.tensor_mul(t2[:], mub2[:], g2za[:])
        nc.vector.tensor_tensor(d01[:], d01[:], t2[:], op=ALU.subtract)
        nc.vector.tensor_tensor(d01[:], pX[:], d01[:], op=ALU.add)
        nc.vector.tensor_mul(d01[:], d01[:], rsta[:])
        nc.vector.tensor_mul(d01[:], d01[:], rstb[:])
        nc.vector.tensor_add(d01[:], d01[:], gbta[:])
        nc.vector.tensor_add(d01[:], d01[:], gbtb[:])
        nc.vector.tensor_scalar_add(d01[:], d01[:], s_bb)

        den = lane_t("den")
        nc.scalar.activation(n2a[:], n2a[:], AF.Sqrt)
        nc.vector.tensor_scalar_max(n2a[:], n2a[:], EPS_COS)
        nc.scalar.activation(n2b[:], n2b[:], AF.Sqrt)
        nc.vector.tensor_scalar_max(n2b[:], n2b[:], EPS_COS)
        nc.vector.tensor_mul(den[:], n2a[:], n2b[:])
        nc.vector.reciprocal(den[:], den[:])
        atg_sl = lane_t("atg_sl")
        nc.vector.tensor_mul(atg_sl[:], d01[:], den[:])
        nc.gpsimd.dma_start(t["out"][1:2, ts(mt, NP)], atg_sl[:])


# ===================== host side =====================

def kernel(**inputs):
    f32 = np.float32
    bf16 = ml_dtypes.bfloat16
    txt = np.ascontiguousarray(
        np.asarray(inputs["text_embeddings"], f32).reshape(S, D))
    cand_full = np.ascontiguousarray(
        np.asarray(inputs["candidate_embeddings"], f32).reshape(M * K, D))
    starts = np.asarray(inputs["mention_starts"], np.int64)
    spans = np.asarray(inputs["span_lengths"], np.int64)
    ends = starts + spans

    j = np.stack([ends + 1, starts,
                  np.minimum(S - 1, ends + CTX),
                  np.maximum(0, starts - CTX)], axis=1)       # [M, 4]
    chunk_of = (np.maximum(j - 1, 0) // P).astype(np.int64)   # [M, 4]
    inv = np.stack([1.0 / (spans + 1).astype(f32),
                    1.0 / (j[:, 2] - j[:, 3]).astype(f32)], axis=1)

    consts = {
        "tri": np.triu(np.ones((P, P), f32)),
        "tri32": np.triu(np.ones((NCH, NCH), f32), k=1),
        "ident": np.eye(P, dtype=f32),
        "identb": np.eye(P, dtype=f32).astype(bf16),
        "zrow": np.zeros((1, D), f32),
        "hmat": np.repeat(np.eye(H, dtype=f32), DH, axis=0).astype(bf16),
        "i8neg": (-np.eye(H, dtype=f32)).astype(bf16),
    }
    wnames = ["relik_w1", "relik_b1", "relik_w2",
              "wq", "bq", "wk", "bk", "wv", "bv", "wo", "bo",
              "ln1_g", "ln1_b", "ffn_w1", "ffn_b1", "ffn_w2", "ffn_b2",
              "ln2_g", "ln2_b", "uni_w1", "uni_b1", "uni_w2"]
    weights = {n: np.ascontiguousarray(np.asarray(inputs[n], f32))
               for n in wnames}
    weights["relik_b2"] = np.asarray(inputs["relik_b2"], f32).reshape(1, 1)
    weights["uni_b2"] = np.ascontiguousarray(
        np.asarray(inputs["uni_b2"], f32).reshape(1, D))

    in_maps = []
    for core in range(NCORES):
        sl = slice(core * M_LOC, (core + 1) * M_LOC)
        selt = np.zeros((NCH, 2, P), f32)
        jc = chunk_of[sl]                                     # [128, 4]
        ar = np.arange(P)
        for col, (tp, tm) in enumerate(((0, 1), (2, 3))):
            np.add.at(selt, (jc[:, tp], col, ar), 1.0)
            np.add.at(selt, (jc[:, tm], col, ar), -1.0)
        im = {
            "txt": txt,
            "cand": cand_full[core * PAIRS:(core + 1) * PAIRS],
            "idx": np.ascontiguousarray(j[sl].astype(np.int32)),
            "invl": np.ascontiguousarray(inv[sl].astype(f32)),
            "seltab": selt,
        }
        im.update(consts)
        im.update(weights)
        in_maps.append(im)

    if "nc" not in _NC_CACHE:
        _NC_CACHE["nc"] = _build_nc()
    nc = _NC_CACHE["nc"]

    results = bass_utils.run_bass_kernel_spmd(
        nc, in_maps, core_ids=list(range(NCORES))).results

    out = np.zeros((3, M, K), f32)
    for core in range(NCORES):
        sl = slice(core * M_LOC, (core + 1) * M_LOC)
        out[:, sl, :] = results[core]["out"].reshape(3, M_LOC, K)
    return out


if __name__ == "__main__":
    nc = _build_nc()
    print("built ok")



# revision 17
# speedup vs baseline: 1.5674x; 1.5674x over previous
"""Trainium2 Bass kernel for nn_EntityResolutionProcessor.

Strategy: data-parallel over mentions (M=1024 -> 128/core on 8 cores).
Host pre-work: candidates transposed to feature-major (bf16 + fp8 copies),
weights scaled x64, converted to fp8e4 (bf16 for relik), laid out
feature-major. Device keeps all pair-side weights resident in SBUF and
runs the heavy per-pair matmuls as fp8 DoubleRow (2x128 contraction per
instruction, 0.5 cycles/row).
On-device per core:
  phase0: blocked cumsum of text -> csum scratch in DRAM (f32); indirect-DMA
          gather of 4 csum rows per mention; mention/context means (f32).
  per-mention: feature-major projections (relik-W1a, q/k/v, uni-U1a), s_aa.
  8 macro-tiles of 512 pairs: k/v/q projections (fp8), relik head (bf16),
          uni head (fp8), 2-token attention via sigmoid softmax, wo (fp8)
          + LN1, FFN (fp8), LN2+cosine fused via sufficient statistics.
Activations feature-major [feat->6x128 partitions, pairs].
"""

from contextlib import ExitStack

import ml_dtypes
import numpy as np

import concourse.bass as bass
import concourse.mybir as mybir
import concourse.tile as tile
from concourse import bacc, bass_isa, bass_utils
from concourse.bass import IndirectOffsetOnAxis, ds, ts

S, D, M, K, H = 4096, 768, 1024, 32, 8
DH = D // H
CTX = 10
NCORES = 8
P = 128
FC = D // P                     # 6 feature chunks
HFC = 4 * D // P                # 24 ffn hidden chunks
M_LOC = M // NCORES             # 128 mentions per core
PAIRS = M_LOC * K               # 4096 pairs per core
NP = 512                        # pairs per macro tile
G = NP // K                     # 16 mentions per macro tile
NMACRO = PAIRS // NP            # 8
NCH = S // P                    # 32 text chunks
ISQ = 1.0 / float(np.sqrt(np.float32(DH)))
EPS_LN = 1e-5
EPS_COS = 1e-8
W8SCALE = 64.0
IW8 = 1.0 / W8SCALE

F32 = mybir.dt.float32
BF16 = mybir.dt.bfloat16
FP8 = mybir.dt.float8e4
I32 = mybir.dt.int32
AF = mybir.ActivationFunctionType
ALU = mybir.AluOpType
DR = mybir.MatmulPerfMode.DoubleRow

_NC_CACHE = {}


def _gk(ap):
    """view a [128, NP] AP as [128, G, K]"""
    return ap.rearrange("p (g k) -> p g k", g=G)


def _feat_major(w_ap):
    """[in, out] dram AP -> [128, in//128, out] (partition = in % 128)"""
    return w_ap.rearrange("(i p) o -> p i o", p=P)


def _vec6(v_ap, n=FC):
    """[D] dram AP -> [128, n] per-feature layout"""
    return v_ap.rearrange("(i p) -> p i", p=P)


def _build_nc():
    nc = bacc.Bacc(
        "TRN2", target_bir_lowering=False, debug=False, num_devices=NCORES
    )

    def inp(name, shape, dtype=F32):
        return nc.dram_tensor(name, list(shape), dtype, kind="ExternalInput").ap()

    t = {}
    t["txt"] = inp("txt", [S, D])
    t["candT_bf"] = inp("candT_bf", [P, FC, PAIRS], BF16)
    t["candT_8"] = inp("candT_8", [P, FC, PAIRS], FP8)
    t["idx"] = inp("idx", [P, 4], I32)
    t["invl"] = inp("invl", [P, 2])
    t["seltab"] = inp("seltab", [NCH, 2, P])
    t["tri"] = inp("tri", [P, P])          # upper-tri incl (lhsT of L)
    t["tri32"] = inp("tri32", [NCH, NCH])  # strict upper (lhsT of strict L)
    t["ident"] = inp("ident", [P, P])
    t["zrow"] = inp("zrow", [1, D])
    t["hmat"] = inp("hmat", [D, H], BF16)  # head indicator
    t["i8neg"] = inp("i8neg", [H, H], BF16)

    # pre-laid-out weights (host: feature-major, x64 scale for fp8)
    t["wk8"] = inp("wk8", [P, FC, D], FP8)
    t["wv8"] = inp("wv8", [P, FC, D], FP8)
    t["wq8"] = inp("wq8", [P, FC, D], FP8)
    t["wo8"] = inp("wo8", [P, FC, D], FP8)
    t["u1b8"] = inp("u1b8", [P, FC, D], FP8)
    t["fw18"] = inp("fw18", [P, FC, 4 * D], FP8)
    t["fw28"] = inp("fw28", [P, HFC, D], FP8)
    t["w1bb"] = inp("w1bb", [P, FC, D], BF16)   # relik W1 candidate half

    for n, shp in [("relik_w1", [2 * D, D]), ("relik_b1", [D]),
                   ("relik_w2", [D, 1]), ("relik_b2", [1, 1]),
                   ("wq", [D, D]), ("bq", [D]), ("wk", [D, D]), ("bk", [D]),
                   ("wv", [D, D]), ("bv", [D]), ("bo", [D]),
                   ("ln1_g", [D]), ("ln1_b", [D]),
                   ("ffn_b1", [4 * D]), ("ffn_b2", [D]),
                   ("ln2_g", [D]), ("ln2_b", [D]),
                   ("uni_w1", [2 * D, D]), ("uni_b1", [D]),
                   ("uni_w2", [D, D]), ("uni_b2", [1, D])]:
        t[n] = inp(n, shp)

    t["out"] = nc.dram_tensor("out", [3, PAIRS], F32, kind="ExternalOutput").ap()
    t["csum"] = nc.dram_tensor("csum_scratch", [S + 1, D], F32).ap()

    with tile.TileContext(nc) as tc:
        _body(nc, tc, t)
    nc.compile()
    return nc


def _body(nc, tc, t):
    with ExitStack() as _ctx:
        _body_inner(nc, tc, t, _ctx)


def _body_inner(nc, tc, t, _ctx):
    mm = lambda *a, **k: nc.tensor.matmul(*a, **k)

    # ---------------- pools ----------------
    psum = _ctx.enter_context(tc.tile_pool(name="psum", bufs=1, space="PSUM"))
    res = _ctx.enter_context(tc.tile_pool(name="res", bufs=1))

    def ps_mm(shape=(P, NP), dtype=F32):
        return psum.tile(list(shape), dtype, tag="mm", bufs=3,
                         padded_shape=[P, NP], name="ps_mm")

    def ps_score():
        return psum.tile([8, NP], F32, tag="score", bufs=1, name="ps_score")

    def ps_stat():
        # stats tile: MM groups land at base partitions 0 and 64
        return psum.tile([P, NP], F32, tag="stat", bufs=2, name="ps_stat")

    def ps_head():
        return psum.tile([1, NP], F32, tag="head", bufs=2, name="ps_head")

    # ---------------- resident constants ----------------
    def load_res(name, ap_src, shape, dtype=F32):
        tl = res.tile(list(shape), dtype, name=name)
        nc.gpsimd.dma_start(tl[:], ap_src)
        return tl

    tri_sb = load_res("tri_sb", t["tri"][:], [P, P])
    tri32_sb = load_res("tri32_sb", t["tri32"][:], [NCH, NCH])
    ident_sb = load_res("ident_sb", t["ident"][:], [P, P])
    i8neg_sb = load_res("i8neg_sb", t["i8neg"][:], [H, H], BF16)
    # H in two layouts: lhsT for head-reduce [128,6,8]; lhsT for bcast [8,6,128]
    h_sb = load_res("h_sb", t["hmat"].rearrange("(c p) h -> p c h", p=P),
                    [P, FC, H], BF16)
    ht_sb = load_res("ht_sb", t["hmat"].rearrange("(c p) h -> h c p", p=P),
                     [H, FC, P], BF16)
    negh_sb = res.tile([P, FC, H], BF16, name="negh_sb")
    nc.vector.tensor_scalar_mul(negh_sb[:], h_sb[:], -1.0)

    idx_sb = load_res("idx_sb", t["idx"][:], [P, 4], I32)
    invl_sb = load_res("invl_sb", t["invl"][:], [P, 2])
    sel_sb = load_res("sel_sb", t["seltab"][:], [NCH, 2, P])

    # resident pair-side weights
    wk8_sb = load_res("wk8_sb", t["wk8"][:], [P, FC, D], FP8)
    wv8_sb = load_res("wv8_sb", t["wv8"][:], [P, FC, D], FP8)
    wq8_sb = load_res("wq8_sb", t["wq8"][:], [P, FC, D], FP8)
    wo8_sb = load_res("wo8_sb", t["wo8"][:], [P, FC, D], FP8)
    u1b8_sb = load_res("u1b8_sb", t["u1b8"][:], [P, FC, D], FP8)
    fw18_sb = load_res("fw18_sb", t["fw18"][:], [P, FC, 4 * D], FP8)
    fw28_sb = load_res("fw28_sb", t["fw28"][:], [P, HFC, D], FP8)
    w1bb_sb = load_res("w1bb_sb", t["w1bb"][:], [P, FC, D], BF16)

    bq_sb = load_res("bq_sb", _vec6(t["bq"]), [P, FC])
    bk_sb = load_res("bk_sb", _vec6(t["bk"]), [P, FC])
    bv_sb = load_res("bv_sb", _vec6(t["bv"]), [P, FC])
    bo_sb = load_res("bo_sb", _vec6(t["bo"]), [P, FC])
    rb1_sb = load_res("rb1_sb", _vec6(t["relik_b1"]), [P, FC])
    ub1_sb = load_res("ub1_sb", _vec6(t["uni_b1"]), [P, FC])
    fb1_sb = load_res("fb1_sb", _vec6(t["ffn_b1"], HFC), [P, HFC])
    fb2_sb = load_res("fb2_sb", _vec6(t["ffn_b2"]), [P, FC])
    l1g_sb = load_res("l1g_sb", _vec6(t["ln1_g"]), [P, FC])
    l1b_sb = load_res("l1b_sb", _vec6(t["ln1_b"]), [P, FC])
    l2g_sb = load_res("l2g_sb", _vec6(t["ln2_g"]), [P, FC])
    l2b_sb = load_res("l2b_sb", _vec6(t["ln2_b"]), [P, FC])
    rw2_sb = load_res("rw2_sb",
                      t["relik_w2"].rearrange("(c p) o -> p c o", p=P),
                      [P, FC, 1], BF16)
    rb2_sb = load_res("rb2_sb", t["relik_b2"][:], [1, 1])

    ones_sb = res.tile([P, 1], BF16, name="ones_sb")
    nc.vector.memset(ones_sb[:], 1.0)
    onesf_sb = res.tile([P, 1], F32, name="onesf_sb")
    nc.vector.memset(onesf_sb[:], 1.0)
    ones_row = res.tile([1, P], BF16, name="ones_row")
    nc.vector.memset(ones_row[:], 1.0)

    # stats lhsT [128, 6, 3]: cols = [1, g2^2, g2*b2] per feature chunk
    sl3_sb = res.tile([P, FC, 3], BF16, name="sl3_sb")
    # sq-stats lhsT [128, 6, 2]: cols = [1, g2^2]
    sl2_sb = res.tile([P, FC, 2], BF16, name="sl2_sb")
    g2sq_sb = res.tile([P, FC], F32, name="g2sq_sb")
    g2b2_sb = res.tile([P, FC], F32, name="g2b2_sb")
    nc.vector.tensor_mul(g2sq_sb[:], l2g_sb[:], l2g_sb[:])
    nc.vector.tensor_mul(g2b2_sb[:], l2g_sb[:], l2b_sb[:])
    for c in range(FC):
        nc.vector.tensor_copy(sl3_sb[:, c, 0:1], ones_sb[:])
        nc.vector.tensor_copy(sl3_sb[:, c, 1:2], g2sq_sb[:, c:c + 1])
        nc.vector.tensor_copy(sl3_sb[:, c, 2:3], g2b2_sb[:, c:c + 1])
        nc.vector.tensor_copy(sl2_sb[:, c, 0:1], ones_sb[:])
        nc.vector.tensor_copy(sl2_sb[:, c, 1:2], g2sq_sb[:, c:c + 1])

    # scalar reductions of bias/gain vectors (each -> [1,1] on partition 0)
    def vec_sum(name, vecs):
        tmp = res.tile([P, FC], F32, name=name + "_t")
        if len(vecs) == 1:
            nc.vector.tensor_copy(tmp[:], vecs[0][:])
        else:
            nc.vector.tensor_mul(tmp[:], vecs[0][:], vecs[1][:])
            for v in vecs[2:]:
                nc.vector.tensor_mul(tmp[:], tmp[:], v[:])
        red = res.tile([P, 1], F32, name=name + "_r")
        nc.vector.tensor_reduce(red[:], tmp[:], axis=mybir.AxisListType.X,
                                op=ALU.add)
        pR = ps_head()
        mm(pR[:, 0:1], red[:], onesf_sb[:], start=True, stop=True)
        arr = res.tile([1, 1], F32, name=name)
        nc.vector.tensor_copy(arr[:], pR[:, 0:1])
        return arr[0:1, 0:1]

    s_bo = vec_sum("s_bo", [bo_sb])
    s_fb2 = vec_sum("s_fb2", [fb2_sb])
    s_g2 = vec_sum("s_g2", [l2g_sb, l2g_sb])
    s_gb = vec_sum("s_gb", [l2g_sb, l2b_sb])
    s_bb = vec_sum("s_bb", [l2b_sb, l2b_sb])
    s_g2f = vec_sum("s_g2f", [l2g_sb, l2g_sb, fb2_sb])
    s_gbf = vec_sum("s_gbf", [l2g_sb, l2b_sb, fb2_sb])

    u2rs_sb = res.tile([P, FC], BF16, name="u2rs_sb")
    b2m_sb = res.tile([1, 1], F32, name="b2m_sb")

    # per-mention outputs (feature-major [128, 6, 128])
    m_T = res.tile([P, FC, P], F32, name="m_T")     # f32: residual source
    m_Tb = res.tile([P, FC, P], BF16, name="m_Tb")  # bf16: matmul rhs
    c_Tb = res.tile([P, FC, P], BF16, name="c_Tb")
    m_q = res.tile([P, FC, P], BF16, name="m_q")
    m_k = res.tile([P, FC, P], BF16, name="m_k")
    m_v = res.tile([P, FC, P], BF16, name="m_v")
    m_relik = res.tile([P, FC, P], BF16, name="m_relik")
    c_uni = res.tile([P, FC, P], BF16, name="c_uni")
    s_aa_sb = res.tile([H, P], BF16, name="s_aa_sb")

    # ================= phase 0: csum + gather =================
    with tc.tile_pool(name="p0", bufs=1) as p0:
        # uni_w2 row-sums (once)
        u2_sb = p0.tile([P, FC, D], F32, name="u2_sb")
        nc.gpsimd.dma_start(u2_sb[:], _feat_major(t["uni_w2"]))
        u2r_f = p0.tile([P, FC], F32, name="u2r_f")
        nc.vector.tensor_reduce(u2r_f[:], u2_sb[:],
                                axis=mybir.AxisListType.X, op=ALU.add)
        nc.vector.tensor_copy(u2rs_sb[:], u2r_f[:])
        ub2_sb = p0.tile([1, D], F32, name="ub2_sb")
        nc.gpsimd.dma_start(ub2_sb[:], t["uni_b2"][:])
        b2r = p0.tile([1, 1], F32, name="b2r")
        nc.vector.tensor_reduce(b2r[:], ub2_sb[:], axis=mybir.AxisListType.X,
                                op=ALU.add)
        nc.scalar.activation(b2m_sb[:], b2r[:], AF.Copy, scale=1.0 / D)

        # ---- cumsum ----
        totals_sb = p0.tile([NCH, D], F32, name="totals_sb")
        nc.gpsimd.dma_start(t["csum"][0:1, :], t["zrow"][:])

        for c in range(NCH):
            txt_c = p0.tile([P, D], F32, tag="txtc", bufs=3, name="txt_c")
            nc.gpsimd.dma_start(txt_c[:], t["txt"][c * P:(c + 1) * P, :])
            pre_sb = p0.tile([P, D], F32, tag="pre", bufs=3, name="pre_sb")
            for half in range(2):
                sl = ds(half * 384, 384)
                pA = ps_mm((P, 384))
                mm(pA[:], tri_sb[:], txt_c[:, sl], start=True, stop=True)
                nc.any.tensor_copy(pre_sb[:, sl], pA[:])
            nc.gpsimd.dma_start(t["csum"][1 + c * P: 1 + (c + 1) * P, :],
                                pre_sb[:])
            nc.gpsimd.dma_start(totals_sb[c:c + 1, :], pre_sb[P - 1:P, :])

        offs_sb = p0.tile([NCH, D], F32, name="offs_sb")
        for half in range(2):
            sl = ds(half * 384, 384)
            pA = ps_mm((NCH, 384))
            mm(pA[:], tri32_sb[:], totals_sb[:, sl], start=True, stop=True)
            nc.any.tensor_copy(offs_sb[:, sl], pA[:])

        # ---- gathers + means ----
        gath = []
        for j in range(4):
            g_t = p0.tile([P, D], F32, tag=f"g{j}", name=f"g_{j}")
            nc.gpsimd.indirect_dma_start(
                out=g_t[:], out_offset=None, in_=t["csum"][:],
                in_offset=IndirectOffsetOnAxis(ap=idx_sb[:, j:j + 1], axis=0),
            )
            gath.append(g_t)

        def mean_tile(out_name, gp, gm, selcol, inv_col):
            o_t = p0.tile([P, D], F32, name=out_name)
            dif = p0.tile([P, D], F32, tag="dif", bufs=2, name="dif")
            nc.vector.tensor_tensor(dif[:], gath[gp][:], gath[gm][:],
                                    op=ALU.subtract)
            for half in range(2):
                sl = ds(half * 384, 384)
                pA = ps_mm((P, 384))
                mm(pA[:], sel_sb[:, selcol, :], offs_sb[:, sl],
                   start=True, stop=True)
                nc.vector.tensor_tensor(o_t[:, sl], pA[:], dif[:, sl],
                                        op=ALU.add)
            nc.vector.tensor_scalar_mul(o_t[:], o_t[:],
                                        invl_sb[:, inv_col:inv_col + 1])
            return o_t

        mention_rm = mean_tile("mention_rm", 0, 1, 0, 0)
        ctx_rm = mean_tile("ctx_rm", 2, 3, 1, 1)

        for src, dstf, dstb in ((mention_rm, m_T, m_Tb),
                                (ctx_rm, None, c_Tb)):
            for fc in range(FC):
                pT = ps_mm((P, P))
                nc.tensor.transpose(pT[:], src[:, ts(fc, P)], ident_sb[:])
                if dstf is not None:
                    nc.vector.tensor_copy(dstf[:, fc, :], pT[:])
                nc.any.tensor_copy(dstb[:, fc, :], pT[:])

    # ================= pools for the main phase =================
    wts = _ctx.enter_context(tc.tile_pool(name="wts", bufs=1))
    act = _ctx.enter_context(tc.tile_pool(name="act", bufs=1))
    # [1, NP] lane values: tensor_tensor with two SBUF operands requires
    # equal base partitions, so every lane tile sits at partition 0 and
    # slots are assigned statically by liveness (reused across stages).
    lane_tiles = [res.tile([1, NP], F32, name=f"lane_{i}") for i in range(15)]
    LANE_SLOTS = {
        "osl_r": 0, "osl_u": 1,
        "mua": 2, "vara": 3, "rstda": 4, "mub": 5, "varb": 6, "rstdb": 7,
        "sza": 2, "g2za": 3, "gbza": 4, "sqa": 5, "g2qa": 6,
        "szb": 7, "g2zb": 8, "gbzb": 9, "sqb": 10, "g2qb": 11,
        "muza": 0, "var2a": 12, "rstd2a": 13,
        "muzb": 1, "var2b": 12, "rstd2b": 10,
        "gbta": 14, "gbtb": 4,
        "n2a": 9, "n2b": 6,
        "d01": 11, "t2": 5, "den": 12, "atg_sl": 5,
    }

    def load_strip_conv(w_fm_ap, oc):
        """one-shot converting load (per-mention phase)"""
        st = wts.tile([P, FC, P], BF16, tag="wstrip", bufs=2, name="w_strip")
        nc.gpsimd.dma_start(st[:], w_fm_ap[:, :, ts(oc, P)])
        return st

    def unit(tag, name, bufs=1, dtype=BF16):
        return act.tile([P, FC, NP], dtype, tag=tag, bufs=bufs, name=name)

    def chunk_t(name):
        return act.tile([P, NP], BF16, tag="tt", bufs=3, name=name)

    # ---------- per-mention projections (bf16, N=128) ----------
    for w_ap, b_sb, out_t, src in (
        (_feat_major(t["wq"]), bq_sb, m_q, m_Tb),
        (_feat_major(t["wk"]), bk_sb, m_k, m_Tb),
        (_feat_major(t["wv"]), bv_sb, m_v, m_Tb),
        (_feat_major(t["relik_w1"][:D]), rb1_sb, m_relik, m_Tb),
        (_feat_major(t["uni_w1"][:D]), ub1_sb, c_uni, c_Tb),
    ):
        for oc in range(FC):
            st = load_strip_conv(w_ap, oc)
            pA = ps_mm((P, P))
            for ic in range(FC):
                mm(pA[:], st[:, ic, :], src[:, ic, :],
                   start=(ic == 0), stop=(ic == FC - 1))
            nc.scalar.activation(out_t[:, oc, :], pA[:], AF.Identity,
                                 bias=b_sb[:, oc:oc + 1])

    # s_aa [8, 128]
    mprod = wts.tile([P, FC, P], BF16, tag="wstrip", bufs=2, name="mprod")
    for c in range(FC):
        nc.vector.tensor_mul(mprod[:, c, :], m_q[:, c, :], m_k[:, c, :])
    pS = ps_score()
    for c in range(FC):
        mm(pS[:, :P], h_sb[:, c, :], mprod[:, c, :],
           start=(c == 0), stop=(c == FC - 1))
    nc.any.tensor_copy(s_aa_sb[:], pS[:, :P])

    # ================= macro-tile loop =================
    for mt in range(NMACRO):
        g0 = mt * G
        gsl = ds(g0, G)

        def lane_t(name):
            return lane_tiles[LANE_SLOTS[name]][:]

        def mview(mt_tile, c):
            """mention-side bcast view [128, G, K]"""
            return mt_tile[:, c, gsl, None].to_broadcast([P, G, K])

        # ---- candidate feature-major slices (host pre-transposed) ----
        candT = unit("candT", "candT", bufs=2)
        nc.gpsimd.dma_start(candT[:], t["candT_bf"][:, :, ts(mt, NP)])
        candT8 = unit("candT8", "candT8", bufs=1, dtype=FP8)
        nc.gpsimd.dma_start(candT8[:], t["candT_8"][:, :, ts(mt, NP)])

        def dr_proj(pA, w8_sb, rhs8, oc, nic=FC):
            """accumulate out-chunk oc = W.T @ rhs via fp8 DoubleRow"""
            for i in range(nic // 2):
                mm(pA[:], w8_sb[:, 2 * i:2 * i + 2, ts(oc, P)],
                   rhs8[:, 2 * i:2 * i + 2, :],
                   start=(i == 0), stop=(i == nic // 2 - 1), perf_mode=DR)

        # ---- k/v projections (fp8) ----
        k_b = unit("B", "k_b")
        v_b = unit("C", "v_b")
        for w8_sb, b_sb, out_t in ((wk8_sb, bk_sb, k_b),
                                   (wv8_sb, bv_sb, v_b)):
            for oc in range(FC):
                pA = ps_mm()
                dr_proj(pA, w8_sb, candT8, oc)
                nc.scalar.activation(out_t[:, oc, :], pA[:], AF.Identity,
                                     bias=b_sb[:, oc:oc + 1], scale=IW8)

        # ---- relik head (bf16) ----
        h_r = unit("hh", "h_r", bufs=2)
        for oc in range(FC):
            pA = ps_mm()
            for ic in range(FC):
                mm(pA[:], w1bb_sb[:, ic, ts(oc, P)], candT[:, ic, :],
                   start=(ic == 0), stop=(ic == FC - 1))
            nc.vector.tensor_tensor(_gk(h_r[:, oc, :]), _gk(pA[:]),
                                    mview(m_relik, oc), op=ALU.add)
            nc.scalar.activation(h_r[:, oc, :], h_r[:, oc, :], AF.Relu)
        pH = ps_head()
        for c in range(FC):
            mm(pH[:], rw2_sb[:, c, :], h_r[:, c, :],
               start=(c == 0), stop=(c == FC - 1))
        osl_r = lane_t("osl_r")
        nc.scalar.activation(osl_r[:], pH[:], AF.Identity, bias=rb2_sb[:])
        nc.gpsimd.dma_start(t["out"][0:1, ts(mt, NP)], osl_r[:])

        # ---- uni head (fp8) ----
        h_u = unit("hh", "h_u", bufs=2)
        for oc in range(FC):
            pA = ps_mm()
            dr_proj(pA, u1b8_sb, candT8, oc)
            nc.vector.scalar_tensor_tensor(
                _gk(h_u[:, oc, :]), _gk(pA[:]), IW8, mview(c_uni, oc),
                op0=ALU.mult, op1=ALU.add)
            nc.scalar.activation(h_u[:, oc, :], h_u[:, oc, :], AF.Relu)
        pH2 = ps_head()
        for c in range(FC):
            mm(pH2[:], u2rs_sb[:, c:c + 1], h_u[:, c, :],
               start=(c == 0), stop=(c == FC - 1))
        osl_u = lane_t("osl_u")
        nc.scalar.activation(osl_u[:], pH2[:], AF.Sigmoid, bias=b2m_sb[:],
                             scale=1.0 / D)
        nc.gpsimd.dma_start(t["out"][2:3, ts(mt, NP)], osl_u[:])

        # ---- attention scores ----
        pAB = ps_score()
        for c in range(FC):
            pr1 = chunk_t("pr1")
            nc.vector.tensor_tensor(_gk(pr1[:]), _gk(k_b[:, c, :]),
                                    mview(m_q, c), op=ALU.mult)
            mm(pAB[:], h_sb[:, c, :], pr1[:], start=(c == 0), stop=False)
        mm(pAB[:], i8neg_sb[:],
           s_aa_sb[:, gsl, None].to_broadcast([H, G, K]),
           start=False, stop=True)
        p_ab = act.tile([H, NP], BF16, tag="p_ab", bufs=1, name="p_ab")
        nc.scalar.activation(p_ab[:], pAB[:], AF.Sigmoid, scale=ISQ)

        pBA = ps_score()
        first = True
        for c in range(FC):
            pQ = ps_mm()
            dr_proj(pQ, wq8_sb, candT8, c)
            q_c = chunk_t("q_c")
            nc.scalar.activation(q_c[:], pQ[:], AF.Identity,
                                 bias=bq_sb[:, c:c + 1], scale=IW8)
            pr2 = chunk_t("pr2")
            nc.vector.tensor_tensor(_gk(pr2[:]), _gk(q_c[:]), mview(m_k, c),
                                    op=ALU.mult)
            mm(pBA[:], h_sb[:, c, :], pr2[:], start=first, stop=False)
            first = False
            pr3 = chunk_t("pr3")
            nc.vector.tensor_mul(pr3[:], q_c[:], k_b[:, c, :])
            mm(pBA[:], negh_sb[:, c, :], pr3[:],
               start=False, stop=(c == FC - 1))
        p_ba = act.tile([H, NP], BF16, tag="p_ba", bufs=1, name="p_ba")
        nc.scalar.activation(p_ba[:], pBA[:], AF.Sigmoid, scale=ISQ)

        # ---- attention outputs (written fp8 for the wo matmul) ----
        o_a = unit("F", "o_a", dtype=FP8)
        o_b = unit("G", "o_b", dtype=FP8)
        for c in range(FC):
            dv = chunk_t("dv")
            nc.vector.tensor_tensor(_gk(dv[:]), _gk(v_b[:, c, :]),
                                    mview(m_v, c), op=ALU.subtract)
            pBC = ps_mm()
            mm(pBC[:], ht_sb[:, c, :], p_ab[:], start=True, stop=True)
            tmp = chunk_t("tmp")
            nc.vector.tensor_mul(tmp[:], pBC[:], dv[:])
            nc.vector.tensor_tensor(_gk(o_a[:, c, :]), _gk(tmp[:]),
                                    mview(m_v, c), op=ALU.add)
            pBC2 = ps_mm()
            mm(pBC2[:], ht_sb[:, c, :], p_ba[:], start=True, stop=True)
            tmp2 = chunk_t("tmp2")
            nc.vector.tensor_mul(tmp2[:], pBC2[:], dv[:])
            nc.vector.tensor_tensor(o_b[:, c, :], v_b[:, c, :], tmp2[:],
                                    op=ALU.subtract)

        # ---- wo + residual (fp8) ----
        r_a = unit("hh", "r_a", bufs=2)
        r_b = unit("hh", "r_b", bufs=2)
        for oc in range(FC):
            pA = ps_mm()
            dr_proj(pA, wo8_sb, o_a, oc)
            nc.vector.scalar_tensor_tensor(
                _gk(r_a[:, oc, :]), _gk(pA[:]), IW8, mview(m_T, oc),
                op0=ALU.mult, op1=ALU.add)
            pB = ps_mm()
            dr_proj(pB, wo8_sb, o_b, oc)
            nc.vector.scalar_tensor_tensor(
                r_b[:, oc, :], pB[:], IW8, candT[:, oc, :],
                op0=ALU.mult, op1=ALU.add)

        # ---- LN1 (general gains) -> x1 ----
        def layernorm1(r_t, x1_t, tok):
            pSt = ps_stat()
            for c in range(FC):
                sq = chunk_t("sq")
                nc.scalar.activation(sq[:], r_t[:, c, :], AF.Square,
                                     bias=bo_sb[:, c:c + 1])
                mm(pSt[0:1, :], ones_sb[:], r_t[:, c, :],
                   start=(c == 0), stop=(c == FC - 1))
                mm(pSt[32:33, :], ones_sb[:], sq[:],
                   start=(c == 0), stop=(c == FC - 1))
            mu = lane_t("mu" + tok)
            nc.vector.tensor_scalar(mu[:], pSt[0:1, :], s_bo, 1.0 / D,
                                    op0=ALU.add, op1=ALU.mult)
            var = lane_t("var" + tok)
            nc.vector.tensor_mul(var[:], mu[:], mu[:])
            nc.vector.scalar_tensor_tensor(var[:], pSt[32:33, :], 1.0 / D,
                                           var[:], op0=ALU.mult,
                                           op1=ALU.subtract)
            rstd = lane_t("rstd" + tok)
            nc.vector.tensor_scalar_add(var[:], var[:], EPS_LN)
            nc.scalar.activation(rstd[:], var[:], AF.Sqrt)
            nc.vector.reciprocal(rstd[:], rstd[:])
            mubf = act.tile([1, NP], BF16, tag="mubf", bufs=1, name="mubf")
            rstdbf = act.tile([1, NP], BF16, tag="rstdbf", bufs=1,
                              name="rstdbf")
            nc.vector.tensor_copy(mubf[:], mu[:])
            nc.vector.tensor_copy(rstdbf[:], rstd[:])
            mu_bc = ps_mm()
            rstd_bc = ps_mm()
            mm(mu_bc[:], ones_row[:], mubf[:], start=True, stop=True)
            mm(rstd_bc[:], ones_row[:], rstdbf[:], start=True, stop=True)
            for c in range(FC):
                nc.vector.tensor_tensor(x1_t[:, c, :], r_t[:, c, :],
                                        mu_bc[:], op=ALU.subtract)
                nc.vector.scalar_tensor_tensor(
                    x1_t[:, c, :], x1_t[:, c, :], bo_sb[:, c:c + 1],
                    rstd_bc[:], op0=ALU.add, op1=ALU.mult)
                nc.vector.tensor_scalar(
                    x1_t[:, c, :], x1_t[:, c, :], l1g_sb[:, c:c + 1],
                    l1b_sb[:, c:c + 1], op0=ALU.mult, op1=ALU.add)

        x1_a = unit("A", "x1_a")
        x1_b = unit("B2", "x1_b")
        layernorm1(r_a, x1_a, "a")
        layernorm1(r_b, x1_b, "b")

        # fp8 copies for FFN rhs (Pool engine; it is mostly idle).
        # o_a/o_b are dead after the wo matmuls; reuse their space.
        x18_a = unit("F", "x18_a", dtype=FP8)
        x18_b = unit("G", "x18_b", dtype=FP8)
        for c in range(FC):
            nc.gpsimd.tensor_copy(x18_a[:, c, :], x1_a[:, c, :])
            nc.gpsimd.tensor_copy(x18_b[:, c, :], x1_b[:, c, :])

        # ---- FFN (fp8 DoubleRow; tokens interleaved) ----
        h_a = act.tile([P, HFC, NP], FP8, tag="h", bufs=1, name="h_a")
        h_b = act.tile([P, HFC, NP], FP8, tag="h2", bufs=1, name="h_b")
        for hc in range(HFC):
            for x18_t, h_t in ((x18_a, h_a), (x18_b, h_b)):
                pA = ps_mm()
                dr_proj(pA, fw18_sb, x18_t, hc)
                nc.scalar.activation(h_t[:, hc, :], pA[:], AF.Relu,
                                     bias=fb1_sb[:, hc:hc + 1], scale=IW8)
        # k_b/v_b are dead after the attention stage; reuse their space
        r2_a = unit("C", "r2_a")
        r2_b = unit("B", "r2_b")
        for oc in range(FC):
            for x1_t, h_t, r2_t in ((x1_a, h_a, r2_a), (x1_b, h_b, r2_b)):
                pA = ps_mm()
                dr_proj(pA, fw28_sb, h_t, oc, nic=HFC)
                nc.vector.scalar_tensor_tensor(
                    r2_t[:, oc, :], pA[:], IW8, x1_t[:, oc, :],
                    op0=ALU.mult, op1=ALU.add)

        # ---- LN2 + cosine via sufficient statistics ----
        def ln2_stats(r2_t, tok):
            pSt = ps_stat()
            pS2 = ps_stat()
            for c in range(FC):
                sq = chunk_t("sq")
                nc.scalar.activation(sq[:], r2_t[:, c, :], AF.Square,
                                     bias=fb2_sb[:, c:c + 1])
                mm(pSt[0:1, :], sl3_sb[:, c, 0:1], r2_t[:, c, :],
                   start=(c == 0), stop=(c == FC - 1))
                mm(pSt[32:33, :], sl3_sb[:, c, 1:2], r2_t[:, c, :],
                   start=(c == 0), stop=(c == FC - 1))
                mm(pSt[64:65, :], sl3_sb[:, c, 2:3], r2_t[:, c, :],
                   start=(c == 0), stop=(c == FC - 1))
                mm(pSt[96:97, :], sl3_sb[:, c, 0:1], sq[:],
                   start=(c == 0), stop=(c == FC - 1),
                   tile_position=(0, 96))
                mm(pS2[0:1, :], sl2_sb[:, c, 1:2], sq[:],
                   start=(c == 0), stop=(c == FC - 1))
            # evict the five stats rows into base-0 lane tiles, folding the
            # constant fb2 corrections
            sz = lane_t("sz" + tok)
            nc.vector.tensor_scalar_add(sz[:], pSt[0:1, :], s_fb2)
            g2z = lane_t("g2z" + tok)
            nc.vector.tensor_scalar_add(g2z[:], pSt[32:33, :], s_g2f)
            gbz = lane_t("gbz" + tok)
            nc.vector.tensor_scalar_add(gbz[:], pSt[64:65, :], s_gbf)
            sq_s = lane_t("sq" + tok)
            nc.vector.tensor_copy(sq_s[:], pSt[96:97, :])
            g2q = lane_t("g2q" + tok)
            nc.vector.tensor_copy(g2q[:], pS2[0:1, :])
            return sz, g2z, gbz, sq_s, g2q

        stats_a = ln2_stats(r2_a, "a")
        stats_b = ln2_stats(r2_b, "b")
        pX = ps_head()
        for c in range(FC):
            rr = chunk_t("rr")
            nc.vector.tensor_scalar_add(rr[:], r2_b[:, c, :],
                                        fb2_sb[:, c:c + 1])
            nc.vector.scalar_tensor_tensor(rr[:], r2_a[:, c, :],
                                           fb2_sb[:, c:c + 1], rr[:],
                                           op0=ALU.add, op1=ALU.mult)
            mm(pX[:], sl3_sb[:, c, 1:2], rr[:],
               start=(c == 0), stop=(c == FC - 1))

        # lane algebra for cosine
        def ln2_lane(stats, tok):
            sz, g2z, gbz, sq_s, g2q = stats
            muz = lane_t("muz" + tok)
            nc.vector.tensor_scalar_mul(muz[:], sz[:], 1.0 / D)
            var = lane_t("var2" + tok)
            nc.vector.tensor_mul(var[:], muz[:], muz[:])
            nc.vector.scalar_tensor_tensor(var[:], sq_s[:], 1.0 / D,
                                           var[:], op0=ALU.mult,
                                           op1=ALU.subtract)
            rstd = lane_t("rstd2" + tok)
            nc.vector.tensor_scalar_add(var[:], var[:], EPS_LN)
            nc.scalar.activation(rstd[:], var[:], AF.Sqrt)
            nc.vector.reciprocal(rstd[:], rstd[:])
            return muz, rstd, g2z, gbz, g2q

        mua, rsta, g2za, gbza, g2qa = ln2_lane(stats_a, "a")
        mub2, rstb, g2zb, gbzb, g2qb = ln2_lane(stats_b, "b")

        def gbt(mu, rstd, gbz, name):
            o_t = lane_t(name)
            nc.vector.tensor_scalar_mul(o_t[:], mu[:], s_gb)
            nc.vector.tensor_tensor(o_t[:], gbz[:], o_t[:], op=ALU.subtract)
            nc.vector.tensor_mul(o_t[:], o_t[:], rstd[:])
            return o_t

        gbta = gbt(mua, rsta, gbza, "gbta")
        gbtb = gbt(mub2, rstb, gbzb, "gbtb")

        def normsq(mu, rstd, g2z, g2q, gbt_t, name):
            o_t = lane_t(name)
            nc.vector.tensor_scalar_mul(o_t[:], mu[:], s_g2)
            nc.vector.scalar_tensor_tensor(o_t[:], g2z[:], -2.0, o_t[:],
                                           op0=ALU.mult, op1=ALU.add)
            nc.vector.tensor_mul(o_t[:], o_t[:], mu[:])
            nc.vector.tensor_add(o_t[:], o_t[:], g2q[:])
            nc.vector.tensor_mul(o_t[:], o_t[:], rstd[:])
            nc.vector.tensor_mul(o_t[:], o_t[:], rstd[:])
            nc.vector.scalar_tensor_tensor(o_t[:], gbt_t[:], 2.0, o_t[:],
                                           op0=ALU.mult, op1=ALU.add)
            nc.vector.tensor_scalar_add(o_t[:], o_t[:], s_bb)
            return o_t

        n2a = normsq(mua, rsta, g2za, g2qa, gbta, "n2a")
        n2b = normsq(mub2, rstb, g2zb, g2qb, gbtb, "n2b")

        d01 = lane_t("d01")
        nc.vector.tensor_scalar_mul(d01[:], mub2[:], s_g2)
        nc.vector.tensor_tensor(d01[:], d01[:], g2zb[:], op=ALU.subtract)
        nc.vector.tensor_mul(d01[:], d01[:], mua[:])
        t2 = lane_t("t2")
        nc.vector.tensor_mul(t2[:], mub2[:], g2za[:])
        nc.vector.tensor_tensor(d01[:], d01[:], t2[:], op=ALU.subtract)
        nc.vector.tensor_tensor(d01[:], pX[:], d01[:], op=ALU.add)
        nc.vector.tensor_mul(d01[:], d01[:], rsta[:])
        nc.vector.tensor_mul(d01[:], d01[:], rstb[:])
        nc.vector.tensor_add(d01[:], d01[:], gbta[:])
        nc.vector.tensor_add(d01[:], d01[:], gbtb[:])
        nc.vector.tensor_scalar_add(d01[:], d01[:], s_bb)

        den = lane_t("den")
        nc.scalar.activation(n2a[:], n2a[:], AF.Sqrt)
        nc.vector.tensor_scalar_max(n2a[:], n2a[:], EPS_COS)
        nc.scalar.activation(n2b[:], n2b[:], AF.Sqrt)
        nc.vector.tensor_scalar_max(n2b[:], n2b[:], EPS_COS)
        nc.vector.tensor_mul(den[:], n2a[:], n2b[:])
        nc.vector.reciprocal(den[:], den[:])
        atg_sl = lane_t("atg_sl")
        nc.vector.tensor_mul(atg_sl[:], d01[:], den[:])
        nc.gpsimd.dma_start(t["out"][1:2, ts(mt, NP)], atg_sl[:])


# ===================== host side =====================

def kernel(**inputs):
    f32 = np.float32
    bf16 = ml_dtypes.bfloat16
    fp8 = ml_dtypes.float8_e4m3
    txt = np.ascontiguousarray(
        np.asarray(inputs["text_embeddings"], f32).reshape(S, D))
    cand_full = np.ascontiguousarray(
        np.asarray(inputs["candidate_embeddings"], f32).reshape(M * K, D))
    starts = np.asarray(inputs["mention_starts"], np.int64)
    spans = np.asarray(inputs["span_lengths"], np.int64)
    ends = starts + spans

    j = np.stack([ends + 1, starts,
                  np.minimum(S - 1, ends + CTX),
                  np.maximum(0, starts - CTX)], axis=1)       # [M, 4]
    chunk_of = (np.maximum(j - 1, 0) // P).astype(np.int64)   # [M, 4]
    inv = np.stack([1.0 / (spans + 1).astype(f32),
                    1.0 / (j[:, 2] - j[:, 3]).astype(f32)], axis=1)

    consts = {
        "tri": np.triu(np.ones((P, P), f32)),
        "tri32": np.triu(np.ones((NCH, NCH), f32), k=1),
        "ident": np.eye(P, dtype=f32),
        "zrow": np.zeros((1, D), f32),
        "hmat": np.repeat(np.eye(H, dtype=f32), DH, axis=0).astype(bf16),
        "i8neg": (-np.eye(H, dtype=f32)).astype(bf16),
    }

    def fm(w, ic):
        """[Din, Dout] -> [P, Din//P, Dout] feature-major"""
        return np.ascontiguousarray(
            w.reshape(ic, P, w.shape[1]).transpose(1, 0, 2))

    wf = {n: np.asarray(inputs[n], f32) for n in
          ["relik_w1", "wq", "wk", "wv", "wo", "ffn_w1", "ffn_w2", "uni_w1"]}
    w8 = {
        "wk8": fm(wf["wk"] * W8SCALE, FC).astype(fp8),
        "wv8": fm(wf["wv"] * W8SCALE, FC).astype(fp8),
        "wq8": fm(wf["wq"] * W8SCALE, FC).astype(fp8),
        "wo8": fm(wf["wo"] * W8SCALE, FC).astype(fp8),
        "u1b8": fm(wf["uni_w1"][D:] * W8SCALE, FC).astype(fp8),
        "fw18": fm(wf["ffn_w1"] * W8SCALE, FC).astype(fp8),
        "fw28": fm(wf["ffn_w2"] * W8SCALE, HFC).astype(fp8),
        "w1bb": fm(wf["relik_w1"][D:], FC).astype(bf16),
    }

    wnames = ["relik_w1", "relik_b1", "relik_w2",
              "wq", "bq", "wk", "bk", "wv", "bv", "bo",
              "ln1_g", "ln1_b", "ffn_b1", "ffn_b2",
              "ln2_g", "ln2_b", "uni_w1", "uni_b1", "uni_w2"]
    weights = {n: np.ascontiguousarray(np.asarray(inputs[n], f32))
               for n in wnames}
    weights["relik_b2"] = np.asarray(inputs["relik_b2"], f32).reshape(1, 1)
    weights["uni_b2"] = np.ascontiguousarray(
        np.asarray(inputs["uni_b2"], f32).reshape(1, D))

    in_maps = []
    for core in range(NCORES):
        sl = slice(core * M_LOC, (core + 1) * M_LOC)
        selt = np.zeros((NCH, 2, P), f32)
        jc = chunk_of[sl]                                     # [128, 4]
        ar = np.arange(P)
        for col, (tp, tm) in enumerate(((0, 1), (2, 3))):
            np.add.at(selt, (jc[:, tp], col, ar), 1.0)
            np.add.at(selt, (jc[:, tm], col, ar), -1.0)
        candT = np.ascontiguousarray(
            cand_full[core * PAIRS:(core + 1) * PAIRS].T
            .reshape(FC, P, PAIRS).transpose(1, 0, 2))        # [P, FC, PAIRS]
        im = {
            "txt": txt,
            "candT_bf": candT.astype(bf16),
            "candT_8": candT.astype(fp8),
            "idx": np.ascontiguousarray(j[sl].astype(np.int32)),
            "invl": np.ascontiguousarray(inv[sl].astype(f32)),
            "seltab": selt,
        }
        im.update(consts)
        im.update(weights)
        im.update(w8)
        in_maps.append(im)

    if "nc" not in _NC_CACHE:
        _NC_CACHE["nc"] = _build_nc()
    nc = _NC_CACHE["nc"]

    results = bass_utils.run_bass_kernel_spmd(
        nc, in_maps, core_ids=list(range(NCORES))).results

    out = np.zeros((3, M, K), f32)
    for core in range(NCORES):
        sl = slice(core * M_LOC, (core + 1) * M_LOC)
        out[:, sl, :] = results[core]["out"].reshape(3, M_LOC, K)
    return out


if __name__ == "__main__":
    nc = _build_nc()
    print("built ok")


# revision 33
# speedup vs baseline: 1.7390x; 1.1095x over previous
"""Trainium2 Bass kernel for nn_EntityResolutionProcessor.

Strategy: data-parallel over mentions (M=1024 -> 128/core on 8 cores).
Host pre-work: candidates transposed to feature-major (bf16 + fp8 copies),
weights scaled x64, converted to fp8e4 (bf16 for relik), laid out
feature-major. Device keeps all pair-side weights resident in SBUF and
runs the heavy per-pair matmuls as fp8 DoubleRow (2x128 contraction per
instruction, 0.5 cycles/row).
On-device per core:
  phase0: blocked cumsum of text -> csum scratch in DRAM (f32); indirect-DMA
          gather of 4 csum rows per mention; mention/context means (f32).
  per-mention: feature-major projections (relik-W1a, q/k/v, uni-U1a), s_aa.
  8 macro-tiles of 512 pairs: k/v/q projections (fp8), relik head (bf16),
          uni head (fp8), 2-token attention via sigmoid softmax, wo (fp8)
          + LN1, FFN (fp8), LN2+cosine fused via sufficient statistics.
Activations feature-major [feat->6x128 partitions, pairs].
"""

from contextlib import ExitStack

import ml_dtypes
import numpy as np

import concourse.bass as bass
import concourse.mybir as mybir
import concourse.tile as tile
from concourse import bacc, bass_isa, bass_utils
from concourse.bass import IndirectOffsetOnAxis, ds, ts

S, D, M, K, H = 4096, 768, 1024, 32, 8
DH = D // H
CTX = 10
NCORES = 8
P = 128
FC = D // P                     # 6 feature chunks
HFC = 4 * D // P                # 24 ffn hidden chunks
M_LOC = M // NCORES             # 128 mentions per core
PAIRS = M_LOC * K               # 4096 pairs per core
NP = 512                        # pairs per macro tile
G = NP // K                     # 16 mentions per macro tile
NMACRO = PAIRS // NP            # 8
NCH = S // P                    # 32 text chunks
ISQ = 1.0 / float(np.sqrt(np.float32(DH)))
EPS_LN = 1e-5
EPS_COS = 1e-8
W8SCALE = 64.0
IW8 = 1.0 / W8SCALE

F32 = mybir.dt.float32
BF16 = mybir.dt.bfloat16
FP8 = mybir.dt.float8e4
I32 = mybir.dt.int32
AF = mybir.ActivationFunctionType
ALU = mybir.AluOpType
DR = mybir.MatmulPerfMode.DoubleRow

_NC_CACHE = {}


def _gk(ap):
    """view a [128, NP] AP as [128, G, K]"""
    return ap.rearrange("p (g k) -> p g k", g=G)


def _feat_major(w_ap):
    """[in, out] dram AP -> [128, in//128, out] (partition = in % 128)"""
    return w_ap.rearrange("(i p) o -> p i o", p=P)


def _vec6(v_ap, n=FC):
    """[D] dram AP -> [128, n] per-feature layout"""
    return v_ap.rearrange("(i p) -> p i", p=P)


def _build_nc():
    nc = bacc.Bacc(
        "TRN2", target_bir_lowering=False, debug=False, num_devices=NCORES
    )

    def inp(name, shape, dtype=F32):
        return nc.dram_tensor(name, list(shape), dtype, kind="ExternalInput").ap()

    t = {}
    t["txt"] = inp("txt", [S, D])
    t["candT_bf"] = inp("candT_bf", [P, FC, PAIRS], BF16)
    t["candT_8"] = inp("candT_8", [P, FC, PAIRS], FP8)
    t["idx"] = inp("idx", [P, 4], I32)
    t["invl"] = inp("invl", [P, 2])
    t["seltab"] = inp("seltab", [NCH, 2, P])
    t["tri"] = inp("tri", [P, P])          # upper-tri incl (lhsT of L)
    t["tri32"] = inp("tri32", [NCH, NCH])  # strict upper (lhsT of strict L)
    t["ident"] = inp("ident", [P, P])
    t["zrow"] = inp("zrow", [1, D])
    t["hmat"] = inp("hmat", [D, H], BF16)  # head indicator
    t["i8neg"] = inp("i8neg", [H, H], BF16)

    # pre-laid-out weights (host: feature-major, x64 scale for fp8)
    t["wk8"] = inp("wk8", [P, FC, D], FP8)
    t["wv8"] = inp("wv8", [P, FC, D], FP8)
    t["wq8"] = inp("wq8", [P, FC, D], FP8)
    t["wo8"] = inp("wo8", [P, FC, D], FP8)
    t["u1b8"] = inp("u1b8", [P, FC, D], FP8)
    t["fw18"] = inp("fw18", [P, FC, 4 * D], FP8)
    t["fw28"] = inp("fw28", [P, HFC, D], FP8)
    t["w1bb"] = inp("w1bb", [P, FC, D], BF16)   # relik W1 candidate half
    t["dg64"] = inp("dg64", [P, FC, P], BF16)   # diag(64*ln1_g) blocks
    t["sconsts"] = inp("sconsts", [P, 8])       # replicated scalar sums
    t["sel16"] = inp("sel16", [16, 16, P], BF16)  # row-select for bcasts

    for n, shp in [("relik_w1", [2 * D, D]), ("relik_b1", [D]),
                   ("relik_w2", [D, 1]), ("relik_b2", [1, 1]),
                   ("wq", [D, D]), ("bq", [D]), ("wk", [D, D]), ("bk", [D]),
                   ("wv", [D, D]), ("bv", [D]), ("bo", [D]),
                   ("ln1_b", [D]),
                   ("ffn_b1", [4 * D]), ("ffn_b2", [D]),
                   ("ln2_g", [D]), ("ln2_b", [D]),
                   ("uni_w1", [2 * D, D]), ("uni_b1", [D]),
                   ("uni_w2", [D, D]), ("uni_b2", [1, D])]:
        t[n] = inp(n, shp)

    t["out"] = nc.dram_tensor("out", [3, PAIRS], F32, kind="ExternalOutput").ap()
    t["csum"] = nc.dram_tensor("csum_scratch", [S + 1, D], F32).ap()

    with tile.TileContext(nc) as tc:
        _body(nc, tc, t)
    nc.compile()
    return nc


def _body(nc, tc, t):
    with ExitStack() as _ctx:
        _body_inner(nc, tc, t, _ctx)


def _body_inner(nc, tc, t, _ctx):
    mm = lambda *a, **k: nc.tensor.matmul(*a, **k)

    # ---------------- pools ----------------
    psum = _ctx.enter_context(tc.tile_pool(name="psum", bufs=1, space="PSUM"))
    res = _ctx.enter_context(tc.tile_pool(name="res", bufs=1))

    def ps_mm(shape=(P, NP), dtype=F32):
        return psum.tile(list(shape), dtype, tag="mm", bufs=3,
                         padded_shape=[P, NP], name="ps_mm")

    def ps_score():
        return psum.tile([8, NP], F32, tag="score", bufs=1, name="ps_score")

    def ps_stat():
        # stats tile: MM groups land at base partitions 0 and 64
        return psum.tile([P, NP], F32, tag="stat", bufs=2, name="ps_stat")

    def ps_head():
        return psum.tile([1, NP], F32, tag="head", bufs=2, name="ps_head")

    # ---------------- resident constants ----------------
    def load_res(name, ap_src, shape, dtype=F32):
        tl = res.tile(list(shape), dtype, name=name)
        nc.gpsimd.dma_start(tl[:], ap_src)
        return tl

    tri_sb = load_res("tri_sb", t["tri"][:], [P, P])
    tri32_sb = load_res("tri32_sb", t["tri32"][:], [NCH, NCH])
    ident_sb = load_res("ident_sb", t["ident"][:], [P, P])
    i8neg_sb = load_res("i8neg_sb", t["i8neg"][:], [H, H], BF16)
    # H in two layouts: lhsT for head-reduce [128,6,8]; lhsT for bcast [8,6,128]
    h_sb = load_res("h_sb", t["hmat"].rearrange("(c p) h -> p c h", p=P),
                    [P, FC, H], BF16)
    ht_sb = load_res("ht_sb", t["hmat"].rearrange("(c p) h -> h c p", p=P),
                     [H, FC, P], BF16)
    negh_sb = res.tile([P, FC, H], BF16, name="negh_sb")
    nc.vector.tensor_scalar_mul(negh_sb[:], h_sb[:], -1.0)

    idx_sb = load_res("idx_sb", t["idx"][:], [P, 4], I32)
    invl_sb = load_res("invl_sb", t["invl"][:], [P, 2])
    sel_sb = load_res("sel_sb", t["seltab"][:], [NCH, 2, P])

    # resident pair-side weights
    wk8_sb = load_res("wk8_sb", t["wk8"][:], [P, FC, D], FP8)
    wv8_sb = load_res("wv8_sb", t["wv8"][:], [P, FC, D], FP8)
    wq8_sb = load_res("wq8_sb", t["wq8"][:], [P, FC, D], FP8)
    wo8_sb = load_res("wo8_sb", t["wo8"][:], [P, FC, D], FP8)
    u1b8_sb = load_res("u1b8_sb", t["u1b8"][:], [P, FC, D], FP8)
    fw18_sb = load_res("fw18_sb", t["fw18"][:], [P, FC, 4 * D], FP8)
    fw28_sb = load_res("fw28_sb", t["fw28"][:], [P, HFC, D], FP8)
    w1bb_sb = load_res("w1bb_sb", t["w1bb"][:], [P, FC, D], BF16)
    dg64_sb = load_res("dg64_sb", t["dg64"][:], [P, FC, P], BF16)
    sel16_sb = load_res("sel16_sb", t["sel16"][:], [16, 16, P], BF16)

    bq_sb = load_res("bq_sb", _vec6(t["bq"]), [P, FC])
    bk_sb = load_res("bk_sb", _vec6(t["bk"]), [P, FC])
    bv_sb = load_res("bv_sb", _vec6(t["bv"]), [P, FC])
    bo_sb = load_res("bo_sb", _vec6(t["bo"]), [P, FC])
    rb1_sb = load_res("rb1_sb", _vec6(t["relik_b1"]), [P, FC])
    ub1_sb = load_res("ub1_sb", _vec6(t["uni_b1"]), [P, FC])
    fb1_sb = load_res("fb1_sb", _vec6(t["ffn_b1"], HFC), [P, HFC])
    fb2_sb = load_res("fb2_sb", _vec6(t["ffn_b2"]), [P, FC])
    l1b_sb = load_res("l1b_sb", _vec6(t["ln1_b"]), [P, FC])
    l2g_sb = load_res("l2g_sb", _vec6(t["ln2_g"]), [P, FC])
    l2b_sb = load_res("l2b_sb", _vec6(t["ln2_b"]), [P, FC])
    rw2_sb = load_res("rw2_sb",
                      t["relik_w2"].rearrange("(c p) o -> p c o", p=P),
                      [P, FC, 1], BF16)
    rb2_sb = load_res("rb2_sb", t["relik_b2"][:], [1, 1])

    ones_sb = res.tile([P, 1], BF16, name="ones_sb")
    nc.vector.memset(ones_sb[:], 1.0)

    # stats lhsT [128, 6, 3]: cols = [1, g2^2, g2*b2] per feature chunk
    sl3_sb = res.tile([P, FC, 3], BF16, name="sl3_sb")
    # sq-stats lhsT [128, 6, 2]: cols = [1, g2^2]
    sl2_sb = res.tile([P, FC, 2], BF16, name="sl2_sb")
    g2sq_sb = res.tile([P, FC], F32, name="g2sq_sb")
    g2b2_sb = res.tile([P, FC], F32, name="g2b2_sb")
    nc.vector.tensor_mul(g2sq_sb[:], l2g_sb[:], l2g_sb[:])
    nc.vector.tensor_mul(g2b2_sb[:], l2g_sb[:], l2b_sb[:])
    for c in range(FC):
        nc.vector.tensor_copy(sl3_sb[:, c, 0:1], ones_sb[:])
        nc.vector.tensor_copy(sl3_sb[:, c, 1:2], g2sq_sb[:, c:c + 1])
        nc.vector.tensor_copy(sl3_sb[:, c, 2:3], g2b2_sb[:, c:c + 1])
        nc.vector.tensor_copy(sl2_sb[:, c, 0:1], ones_sb[:])
        nc.vector.tensor_copy(sl2_sb[:, c, 1:2], g2sq_sb[:, c:c + 1])

    # scalar reductions of bias/gain vectors: host-computed, replicated
    # across partitions so both row ops ([1,1] ptr) and transposed ops
    # ([P,1] ptr) can use them
    sc_sb = load_res("sc_sb", t["sconsts"][:], [P, 8])
    SC = {"s_bo": 0, "s_fb2": 1, "s_g2": 2, "s_gb": 3, "s_bb": 4,
          "s_g2f": 5, "s_gbf": 6}

    def sc_row(name):
        return sc_sb[0:1, SC[name]:SC[name] + 1]

    def sc_col(name):
        return sc_sb[:, SC[name]:SC[name] + 1]

    u2rs_sb = res.tile([P, FC], BF16, name="u2rs_sb")
    b2m_sb = res.tile([1, 1], F32, name="b2m_sb")

    # per-mention outputs (feature-major [128, 6, 128])
    m_T = res.tile([P, FC, P], F32, name="m_T")     # f32: residual source
    m_Tb = res.tile([P, FC, P], BF16, name="m_Tb")  # bf16: matmul rhs
    c_Tb = res.tile([P, FC, P], BF16, name="c_Tb")
    m_q = res.tile([P, FC, P], BF16, name="m_q")
    m_k = res.tile([P, FC, P], BF16, name="m_k")
    m_v = res.tile([P, FC, P], BF16, name="m_v")
    m_relik = res.tile([P, FC, P], BF16, name="m_relik")
    c_uni = res.tile([P, FC, P], BF16, name="c_uni")
    s_aa_sb = res.tile([H, P], BF16, name="s_aa_sb")

    # ================= phase 0: csum + gather =================
    with tc.tile_pool(name="p0", bufs=1) as p0:
        # uni_w2 row-sums (once)
        u2_sb = p0.tile([P, FC, D], F32, name="u2_sb")
        nc.gpsimd.dma_start(u2_sb[:], _feat_major(t["uni_w2"]))
        u2r_f = p0.tile([P, FC], F32, name="u2r_f")
        nc.vector.tensor_reduce(u2r_f[:], u2_sb[:],
                                axis=mybir.AxisListType.X, op=ALU.add)
        nc.vector.tensor_copy(u2rs_sb[:], u2r_f[:])
        ub2_sb = p0.tile([1, D], F32, name="ub2_sb")
        nc.gpsimd.dma_start(ub2_sb[:], t["uni_b2"][:])
        b2r = p0.tile([1, 1], F32, name="b2r")
        nc.vector.tensor_reduce(b2r[:], ub2_sb[:], axis=mybir.AxisListType.X,
                                op=ALU.add)
        nc.scalar.activation(b2m_sb[:], b2r[:], AF.Copy, scale=1.0 / D)

        # ---- cumsum ----
        totals_sb = p0.tile([NCH, D], F32, name="totals_sb")
        nc.gpsimd.dma_start(t["csum"][0:1, :], t["zrow"][:])

        for c in range(NCH):
            txt_c = p0.tile([P, D], F32, tag="txtc", bufs=3, name="txt_c")
            nc.gpsimd.dma_start(txt_c[:], t["txt"][c * P:(c + 1) * P, :])
            pre_sb = p0.tile([P, D], F32, tag="pre", bufs=3, name="pre_sb")
            for half in range(2):
                sl = ds(half * 384, 384)
                pA = ps_mm((P, 384))
                mm(pA[:], tri_sb[:], txt_c[:, sl], start=True, stop=True)
                nc.any.tensor_copy(pre_sb[:, sl], pA[:])
            nc.gpsimd.dma_start(t["csum"][1 + c * P: 1 + (c + 1) * P, :],
                                pre_sb[:])
            nc.gpsimd.dma_start(totals_sb[c:c + 1, :], pre_sb[P - 1:P, :])

        offs_sb = p0.tile([NCH, D], F32, name="offs_sb")
        for half in range(2):
            sl = ds(half * 384, 384)
            pA = ps_mm((NCH, 384))
            mm(pA[:], tri32_sb[:], totals_sb[:, sl], start=True, stop=True)
            nc.any.tensor_copy(offs_sb[:, sl], pA[:])

        # ---- gathers + means ----
        gath = []
        for j in range(4):
            g_t = p0.tile([P, D], F32, tag=f"g{j}", name=f"g_{j}")
            nc.gpsimd.indirect_dma_start(
                out=g_t[:], out_offset=None, in_=t["csum"][:],
                in_offset=IndirectOffsetOnAxis(ap=idx_sb[:, j:j + 1], axis=0),
            )
            gath.append(g_t)

        def mean_tile(out_name, gp, gm, selcol, inv_col):
            o_t = p0.tile([P, D], F32, name=out_name)
            dif = p0.tile([P, D], F32, tag="dif", bufs=2, name="dif")
            nc.vector.tensor_tensor(dif[:], gath[gp][:], gath[gm][:],
                                    op=ALU.subtract)
            for half in range(2):
                sl = ds(half * 384, 384)
                pA = ps_mm((P, 384))
                mm(pA[:], sel_sb[:, selcol, :], offs_sb[:, sl],
                   start=True, stop=True)
                nc.vector.tensor_tensor(o_t[:, sl], pA[:], dif[:, sl],
                                        op=ALU.add)
            nc.vector.tensor_scalar_mul(o_t[:], o_t[:],
                                        invl_sb[:, inv_col:inv_col + 1])
            return o_t

        mention_rm = mean_tile("mention_rm", 0, 1, 0, 0)
        ctx_rm = mean_tile("ctx_rm", 2, 3, 1, 1)

        for src, dstf, dstb in ((mention_rm, m_T, m_Tb),
                                (ctx_rm, None, c_Tb)):
            for fc in range(FC):
                pT = ps_mm((P, P))
                nc.tensor.transpose(pT[:], src[:, ts(fc, P)], ident_sb[:])
                if dstf is not None:
                    nc.vector.tensor_copy(dstf[:, fc, :], pT[:])
                nc.any.tensor_copy(dstb[:, fc, :], pT[:])

    # ================= pools for the main phase =================
    wts = _ctx.enter_context(tc.tile_pool(name="wts", bufs=1))
    act = _ctx.enter_context(tc.tile_pool(name="act", bufs=1))
    # row-major [1, NP] tiles for the relik/unirel output slices
    osl_tiles = {n: res.tile([1, NP], F32, name=n)
                 for n in ("osl_r", "osl_u")}

    def load_strip_conv(w_fm_ap, oc):
        """one-shot converting load (per-mention phase)"""
        st = wts.tile([P, FC, P], BF16, tag="wstrip", bufs=2, name="w_strip")
        nc.gpsimd.dma_start(st[:], w_fm_ap[:, :, ts(oc, P)])
        return st

    def unit(tag, name, bufs=1, dtype=BF16):
        return act.tile([P, FC, NP], dtype, tag=tag, bufs=bufs, name=name)

    def chunk_t(name):
        return act.tile([P, NP], BF16, tag="tt", bufs=3, name=name)

    # ---------- per-mention projections (bf16, N=128) ----------
    for w_ap, b_sb, out_t, src in (
        (_feat_major(t["wq"]), bq_sb, m_q, m_Tb),
        (_feat_major(t["wk"]), bk_sb, m_k, m_Tb),
        (_feat_major(t["wv"]), bv_sb, m_v, m_Tb),
        (_feat_major(t["relik_w1"][:D]), rb1_sb, m_relik, m_Tb),
        (_feat_major(t["uni_w1"][:D]), ub1_sb, c_uni, c_Tb),
    ):
        for oc in range(FC):
            st = load_strip_conv(w_ap, oc)
            pA = ps_mm((P, P))
            for ic in range(FC):
                mm(pA[:], st[:, ic, :], src[:, ic, :],
                   start=(ic == 0), stop=(ic == FC - 1))
            nc.scalar.activation(out_t[:, oc, :], pA[:], AF.Identity,
                                 bias=b_sb[:, oc:oc + 1])

    # s_aa [8, 128]
    mprod = wts.tile([P, FC, P], BF16, tag="wstrip", bufs=2, name="mprod")
    for c in range(FC):
        nc.vector.tensor_mul(mprod[:, c, :], m_q[:, c, :], m_k[:, c, :])
    pS = ps_score()
    for c in range(FC):
        mm(pS[:, :P], h_sb[:, c, :], mprod[:, c, :],
           start=(c == 0), stop=(c == FC - 1))
    nc.any.tensor_copy(s_aa_sb[:], pS[:, :P])

    # ================= macro-tile loop =================
    for mt in range(NMACRO):
        g0 = mt * G
        gsl = ds(g0, G)

        def lane_t(name):
            return osl_tiles[name][:]

        def mview(mt_tile, c):
            """mention-side bcast view [128, G, K]"""
            return mt_tile[:, c, gsl, None].to_broadcast([P, G, K])

        # ---- candidate feature-major slices (host pre-transposed) ----
        candT = unit("candT", "candT", bufs=2)
        nc.gpsimd.dma_start(candT[:], t["candT_bf"][:, :, ts(mt, NP)])
        candT8 = unit("candT8", "candT8", bufs=1, dtype=FP8)
        nc.gpsimd.dma_start(candT8[:], t["candT_8"][:, :, ts(mt, NP)])

        def dr_proj(pA, w8_sb, rhs8, oc, nic=FC, stop_last=True):
            """accumulate out-chunk oc = W.T @ rhs via fp8 DoubleRow"""
            for i in range(nic // 2):
                mm(pA[:], w8_sb[:, 2 * i:2 * i + 2, ts(oc, P)],
                   rhs8[:, 2 * i:2 * i + 2, :],
                   start=(i == 0),
                   stop=(stop_last and i == nic // 2 - 1), perf_mode=DR)

        # ---- k/v projections (fp8) ----
        k_b = unit("B", "k_b")
        v_b = unit("C", "v_b")
        for w8_sb, b_sb, out_t in ((wk8_sb, bk_sb, k_b),
                                   (wv8_sb, bv_sb, v_b)):
            for oc in range(FC):
                pA = ps_mm()
                dr_proj(pA, w8_sb, candT8, oc)
                nc.scalar.activation(out_t[:, oc, :], pA[:], AF.Identity,
                                     bias=b_sb[:, oc:oc + 1], scale=IW8)

        # ---- relik head (bf16) ----
        h_r = unit("hh", "h_r", bufs=2)
        for oc in range(FC):
            pA = ps_mm()
            for ic in range(FC):
                mm(pA[:], w1bb_sb[:, ic, ts(oc, P)], candT[:, ic, :],
                   start=(ic == 0), stop=(ic == FC - 1))
            nc.vector.tensor_tensor(_gk(h_r[:, oc, :]), _gk(pA[:]),
                                    mview(m_relik, oc), op=ALU.add)
            nc.scalar.activation(h_r[:, oc, :], h_r[:, oc, :], AF.Relu)
        pH = ps_head()
        for c in range(FC):
            mm(pH[:], rw2_sb[:, c, :], h_r[:, c, :],
               start=(c == 0), stop=(c == FC - 1))
        osl_r = lane_t("osl_r")
        nc.scalar.activation(osl_r[:], pH[:], AF.Identity, bias=rb2_sb[:])
        nc.gpsimd.dma_start(t["out"][0:1, ts(mt, NP)], osl_r[:])

        # ---- uni head (fp8) ----
        h_u = unit("hh", "h_u", bufs=2)
        for oc in range(FC):
            pA = ps_mm()
            dr_proj(pA, u1b8_sb, candT8, oc)
            nc.vector.scalar_tensor_tensor(
                _gk(h_u[:, oc, :]), _gk(pA[:]), IW8, mview(c_uni, oc),
                op0=ALU.mult, op1=ALU.add)
            nc.scalar.activation(h_u[:, oc, :], h_u[:, oc, :], AF.Relu)
        pH2 = ps_head()
        for c in range(FC):
            mm(pH2[:], u2rs_sb[:, c:c + 1], h_u[:, c, :],
               start=(c == 0), stop=(c == FC - 1))
        osl_u = lane_t("osl_u")
        nc.scalar.activation(osl_u[:], pH2[:], AF.Sigmoid, bias=b2m_sb[:],
                             scale=1.0 / D)
        nc.gpsimd.dma_start(t["out"][2:3, ts(mt, NP)], osl_u[:])

        # ---- attention scores ----
        pAB = ps_score()
        for c in range(FC):
            pr1 = chunk_t("pr1")
            nc.vector.tensor_tensor(_gk(pr1[:]), _gk(k_b[:, c, :]),
                                    mview(m_q, c), op=ALU.mult)
            mm(pAB[:], h_sb[:, c, :], pr1[:], start=(c == 0), stop=False)
        mm(pAB[:], i8neg_sb[:],
           s_aa_sb[:, gsl, None].to_broadcast([H, G, K]),
           start=False, stop=True)
        p_ab = act.tile([H, NP], BF16, tag="p_ab", bufs=1, name="p_ab")
        nc.scalar.activation(p_ab[:], pAB[:], AF.Sigmoid, scale=ISQ)

        pBA = ps_score()
        first = True
        for c in range(FC):
            pQ = ps_mm()
            dr_proj(pQ, wq8_sb, candT8, c)
            q_c = chunk_t("q_c")
            nc.scalar.activation(q_c[:], pQ[:], AF.Identity,
                                 bias=bq_sb[:, c:c + 1], scale=IW8)
            pr2 = chunk_t("pr2")
            nc.vector.tensor_tensor(_gk(pr2[:]), _gk(q_c[:]), mview(m_k, c),
                                    op=ALU.mult)
            mm(pBA[:], h_sb[:, c, :], pr2[:], start=first, stop=False)
            first = False
            pr3 = chunk_t("pr3")
            nc.vector.tensor_mul(pr3[:], q_c[:], k_b[:, c, :])
            mm(pBA[:], negh_sb[:, c, :], pr3[:],
               start=False, stop=(c == FC - 1))
        p_ba = act.tile([H, NP], BF16, tag="p_ba", bufs=1, name="p_ba")
        nc.scalar.activation(p_ba[:], pBA[:], AF.Sigmoid, scale=ISQ)

        # ---- attention outputs (written fp8 for the wo matmul) ----
        o_a = unit("F", "o_a", dtype=FP8)
        o_b = unit("G", "o_b", dtype=FP8)
        for c in range(FC):
            dv = chunk_t("dv")
            nc.vector.tensor_tensor(_gk(dv[:]), _gk(v_b[:, c, :]),
                                    mview(m_v, c), op=ALU.subtract)
            pBC = ps_mm()
            mm(pBC[:], ht_sb[:, c, :], p_ab[:], start=True, stop=True)
            tmp = chunk_t("tmp")
            nc.vector.tensor_mul(tmp[:], pBC[:], dv[:])
            nc.vector.tensor_tensor(_gk(o_a[:, c, :]), _gk(tmp[:]),
                                    mview(m_v, c), op=ALU.add)
            pBC2 = ps_mm()
            mm(pBC2[:], ht_sb[:, c, :], p_ba[:], start=True, stop=True)
            tmp2 = chunk_t("tmp2")
            nc.vector.tensor_mul(tmp2[:], pBC2[:], dv[:])
            nc.vector.tensor_tensor(o_b[:, c, :], v_b[:, c, :], tmp2[:],
                                    op=ALU.subtract)

        # ---- wo + residual (fp8) ----
        r_a = unit("hh", "r_a", bufs=2)
        r_b = unit("hh", "r_b", bufs=2)
        for oc in range(FC):
            pA = ps_mm()
            dr_proj(pA, wo8_sb, o_a, oc)
            nc.vector.scalar_tensor_tensor(
                _gk(r_a[:, oc, :]), _gk(pA[:]), IW8, mview(m_T, oc),
                op0=ALU.mult, op1=ALU.add)
            pB = ps_mm()
            dr_proj(pB, wo8_sb, o_b, oc)
            nc.vector.scalar_tensor_tensor(
                r_b[:, oc, :], pB[:], IW8, candT[:, oc, :],
                op0=ALU.mult, op1=ALU.add)

        # ---- LN1 -> x-hat (gain/bias folded into fw18/fb1 and dg64) ----
        # stats land in staging rows, get PE-transposed, and the lane
        # algebra runs as cheap [128, 4] ops (cost-model charges by free
        # size, so [1, 512] row ops are 5x more expensive)
        def small(name):
            return act.tile([P, 4], F32, tag=name, bufs=1, name=name)

        stgL = act.tile([P, NP], F32, tag="stgL", bufs=1, name="stgL")
        for ti, r_t in enumerate((r_a, r_b)):
            pSt = ps_stat()
            for c in range(FC):
                sq = chunk_t("sq")
                nc.scalar.activation(sq[:], r_t[:, c, :], AF.Square,
                                     bias=bo_sb[:, c:c + 1])
                mm(pSt[0:1, :], ones_sb[:], r_t[:, c, :],
                   start=(c == 0), stop=(c == FC - 1))
                mm(pSt[32:33, :], ones_sb[:], sq[:],
                   start=(c == 0), stop=(c == FC - 1))
            nc.vector.tensor_copy(stgL[64 * ti:64 * ti + 1, :], pSt[0:1, :])
            nc.vector.tensor_copy(stgL[64 * ti + 32:64 * ti + 33, :],
                                  pSt[32:33, :])
        pTL = ps_mm()
        for jb in range(4):
            nc.tensor.transpose(pTL[:, ts(jb, P)], stgL[:, ts(jb, P)],
                                ident_sb[:])
        cL = pTL.rearrange("p (j s) -> p j s", s=P)

        # [128, 4] lane algebra; mr packs mu/rstd columns for both tokens
        mr = act.tile([P, 16], F32, tag="mr", bufs=1, name="mr")
        for ti in range(2):
            mu = mr[:, 8 * ti:8 * ti + 4]
            rstd = mr[:, 8 * ti + 4:8 * ti + 8]
            nc.vector.tensor_scalar(mu, cL[:, :, 64 * ti], sc_col("s_bo"),
                                    1.0 / D, op0=ALU.add, op1=ALU.mult)
            var = small(f"varL{ti}")
            nc.vector.tensor_mul(var[:], mu, mu)
            nc.vector.scalar_tensor_tensor(var[:], cL[:, :, 64 * ti + 32],
                                           1.0 / D, var[:], op0=ALU.mult,
                                           op1=ALU.subtract)
            nc.vector.tensor_scalar_add(var[:], var[:], EPS_LN)
            nc.scalar.activation(rstd, var[:], AF.Sqrt)
            nc.vector.reciprocal(rstd, rstd)
        pMr = ps_stat()
        nc.tensor.transpose(pMr[0:16, 0:P], mr[:], ident_sb[:])
        mrt = act.tile([16, P], BF16, tag="mrt", bufs=1, name="mrt")
        nc.vector.tensor_copy(mrt[:], pMr[0:16, 0:P])

        x1_a = unit("A", "x1_a")
        x1_b = unit("B2", "x1_b")
        for ti, (r_t, x1_t) in enumerate(((r_a, x1_a), (r_b, x1_b))):
            mu_bc = ps_mm()
            rstd_bc = ps_mm()
            for jb in range(4):
                mm(mu_bc[:, ts(jb, P)], sel16_sb[:, 8 * ti + jb, :], mrt[:],
                   start=True, stop=True)
                mm(rstd_bc[:, ts(jb, P)], sel16_sb[:, 8 * ti + 4 + jb, :],
                   mrt[:], start=True, stop=True)
            for c in range(FC):
                nc.vector.tensor_tensor(x1_t[:, c, :], r_t[:, c, :],
                                        mu_bc[:], op=ALU.subtract)
                nc.vector.scalar_tensor_tensor(
                    x1_t[:, c, :], x1_t[:, c, :], bo_sb[:, c:c + 1],
                    rstd_bc[:], op0=ALU.add, op1=ALU.mult)

        # fp8 copies for FFN rhs (Pool engine; it is mostly idle).
        # o_a/o_b are dead after the wo matmuls; reuse their space.
        x18_a = unit("F", "x18_a", dtype=FP8)
        x18_b = unit("G", "x18_b", dtype=FP8)
        for c in range(FC):
            nc.gpsimd.tensor_copy(x18_a[:, c, :], x1_a[:, c, :])
            nc.gpsimd.tensor_copy(x18_b[:, c, :], x1_b[:, c, :])

        # ---- FFN (fp8 DoubleRow; tokens interleaved) ----
        h_a = act.tile([P, HFC, NP], FP8, tag="h", bufs=1, name="h_a")
        h_b = act.tile([P, HFC, NP], FP8, tag="h2", bufs=1, name="h_b")
        for hc in range(HFC):
            for x18_t, h_t in ((x18_a, h_a), (x18_b, h_b)):
                pA = ps_mm()
                dr_proj(pA, fw18_sb, x18_t, hc)
                nc.scalar.activation(h_t[:, hc, :], pA[:], AF.Relu,
                                     bias=fb1_sb[:, hc:hc + 1], scale=IW8)
        # k_b/v_b are dead after the attention stage; reuse their space
        r2_a = unit("C", "r2_a")
        r2_b = unit("B", "r2_b")
        for oc in range(FC):
            for x1_t, h_t, r2_t in ((x1_a, h_a, r2_a), (x1_b, h_b, r2_b)):
                pA = ps_mm()
                dr_proj(pA, fw28_sb, h_t, oc, nic=HFC, stop_last=False)
                # residual: 64*ln1_g (x) x-hat added via diag matmul, ln1_b
                # via the activation bias -> eviction runs on Act, not DVE
                mm(pA[:], dg64_sb[:, oc, :], x1_t[:, oc, :],
                   start=False, stop=True)
                nc.scalar.activation(r2_t[:, oc, :], pA[:], AF.Identity,
                                     bias=l1b_sb[:, oc:oc + 1], scale=IW8)

        # ---- LN2 + cosine via sufficient statistics (transposed lanes) ----
        # per token: merged stat groups (3 rows from r2 via sl3, 2 rows from
        # sq via sl2); pX in its own head group. All rows go to 32-aligned
        # staging slots, one PE transpose turns them into [128, 4] columns.
        stg2 = act.tile([P, NP], F32, tag="stg2", bufs=1, name="stg2")
        for ti, r2_t in enumerate(((r2_a), (r2_b))):
            pSt = ps_stat()
            for c in range(FC):
                sq = chunk_t("sq")
                nc.scalar.activation(sq[:], r2_t[:, c, :], AF.Square,
                                     bias=fb2_sb[:, c:c + 1])
                mm(pSt[0:3, :], sl3_sb[:, c, 0:3], r2_t[:, c, :],
                   start=(c == 0), stop=(c == FC - 1))
                mm(pSt[64:66, :], sl2_sb[:, c, 0:2], sq[:],
                   start=(c == 0), stop=(c == FC - 1),
                   tile_position=(0, 64))
            nc.vector.tensor_copy(stg2[64 * ti:64 * ti + 3, :], pSt[0:3, :])
            nc.vector.tensor_copy(stg2[64 * ti + 32:64 * ti + 34, :],
                                  pSt[64:66, :])

        pX = ps_head()
        for c in range(FC):
            rr = chunk_t("rr")
            nc.vector.tensor_scalar_add(rr[:], r2_b[:, c, :],
                                        fb2_sb[:, c:c + 1])
            nc.vector.scalar_tensor_tensor(rr[:], r2_a[:, c, :],
                                           fb2_sb[:, c:c + 1], rr[:],
                                           op0=ALU.add, op1=ALU.mult)
            mm(pX[:], sl3_sb[:, c, 1:2], rr[:],
               start=(c == 0), stop=(c == FC - 1))
        stgX = act.tile([P, NP], F32, tag="stgX", bufs=1, name="stgX")
        nc.vector.tensor_copy(stgX[0:1, :], pX[:])

        pT2 = ps_mm()
        pTX = ps_stat()
        for jb in range(4):
            nc.tensor.transpose(pT2[:, ts(jb, P)], stg2[:, ts(jb, P)],
                                ident_sb[:])
            nc.tensor.transpose(pTX[:, ts(jb, P)], stgX[:, ts(jb, P)],
                                ident_sb[:])
        c2 = pT2.rearrange("p (j s) -> p j s", s=P)
        cX = pTX.rearrange("p (j s) -> p j s", s=P)
        # column s of token ti: sz=+0, g2z=+1, gbz=+2, sq=+32, g2q=+33
        B0 = [0, 64]

        def ln2_lane(ti):
            b = B0[ti]
            muz = small(f"muz{ti}")
            nc.vector.tensor_scalar(muz[:], c2[:, :, b], sc_col("s_fb2"),
                                    1.0 / D, op0=ALU.add, op1=ALU.mult)
            g2z = small(f"g2z{ti}")
            nc.vector.tensor_scalar_add(g2z[:], c2[:, :, b + 1],
                                        sc_col("s_g2f"))
            var = small(f"var2{ti}")
            nc.vector.tensor_mul(var[:], muz[:], muz[:])
            nc.vector.scalar_tensor_tensor(var[:], c2[:, :, b + 32],
                                           1.0 / D, var[:], op0=ALU.mult,
                                           op1=ALU.subtract)
            rstd = small(f"rstd2{ti}")
            nc.vector.tensor_scalar_add(var[:], var[:], EPS_LN)
            nc.scalar.activation(rstd[:], var[:], AF.Sqrt)
            nc.vector.reciprocal(rstd[:], rstd[:])
            # gbt = ((gbz + s_gbf) - muz*s_gb) * rstd
            gbt = small(f"gbt{ti}")
            nc.vector.tensor_scalar_mul(gbt[:], muz[:], sc_col("s_gb"))
            nc.vector.scalar_tensor_tensor(gbt[:], c2[:, :, b + 2],
                                           sc_col("s_gbf"), gbt[:],
                                           op0=ALU.add, op1=ALU.subtract)
            nc.vector.tensor_mul(gbt[:], gbt[:], rstd[:])
            # n2 = ((muz*s_g2 - 2*g2z)*muz + g2q)*rstd^2 + 2*gbt + s_bb
            n2 = small(f"n2{ti}")
            nc.vector.tensor_scalar_mul(n2[:], muz[:], sc_col("s_g2"))
            nc.vector.scalar_tensor_tensor(n2[:], g2z[:], -2.0, n2[:],
                                           op0=ALU.mult, op1=ALU.add)
            nc.vector.tensor_mul(n2[:], n2[:], muz[:])
            nc.vector.tensor_tensor(n2[:], n2[:], c2[:, :, b + 33],
                                    op=ALU.add)
            nc.vector.tensor_mul(n2[:], n2[:], rstd[:])
            nc.vector.tensor_mul(n2[:], n2[:], rstd[:])
            nc.vector.scalar_tensor_tensor(n2[:], gbt[:], 2.0, n2[:],
                                           op0=ALU.mult, op1=ALU.add)
            nc.vector.tensor_scalar_add(n2[:], n2[:], sc_col("s_bb"))
            return muz, rstd, g2z, gbt, n2

        muza, rsta, g2za, gbta, n2a = ln2_lane(0)
        muzb, rstb, g2zb, gbtb, n2b = ln2_lane(1)

        d01 = small("d01")
        nc.vector.tensor_scalar_mul(d01[:], muzb[:], sc_col("s_g2"))
        nc.vector.tensor_tensor(d01[:], d01[:], g2zb[:], op=ALU.subtract)
        nc.vector.tensor_mul(d01[:], d01[:], muza[:])
        t2 = small("t2")
        nc.vector.tensor_mul(t2[:], muzb[:], g2za[:])
        nc.vector.tensor_tensor(d01[:], d01[:], t2[:], op=ALU.subtract)
        nc.vector.tensor_tensor(d01[:], cX[:, :, 0], d01[:], op=ALU.add)
        nc.vector.tensor_mul(d01[:], d01[:], rsta[:])
        nc.vector.tensor_mul(d01[:], d01[:], rstb[:])
        nc.vector.tensor_add(d01[:], d01[:], gbta[:])
        nc.vector.tensor_add(d01[:], d01[:], gbtb[:])
        nc.vector.tensor_scalar_add(d01[:], d01[:], sc_col("s_bb"))

        den = small("den")
        nc.scalar.activation(n2a[:], n2a[:], AF.Sqrt)
        nc.vector.tensor_scalar_max(n2a[:], n2a[:], EPS_COS)
        nc.scalar.activation(n2b[:], n2b[:], AF.Sqrt)
        nc.vector.tensor_scalar_max(n2b[:], n2b[:], EPS_COS)
        nc.vector.tensor_mul(den[:], n2a[:], n2b[:])
        nc.vector.reciprocal(den[:], den[:])
        atg_sl = small("atg_sl")
        nc.vector.tensor_mul(atg_sl[:], d01[:], den[:])
        # pair index = jb*128 + p: write with a transposing DRAM AP
        nc.gpsimd.dma_start(
            t["out"][1:2, ts(mt, NP)].rearrange("o (j p) -> o p j", p=P),
            atg_sl[:])


# ===================== host side =====================

def kernel(**inputs):
    f32 = np.float32
    bf16 = ml_dtypes.bfloat16
    fp8 = ml_dtypes.float8_e4m3
    txt = np.ascontiguousarray(
        np.asarray(inputs["text_embeddings"], f32).reshape(S, D))
    cand_full = np.ascontiguousarray(
        np.asarray(inputs["candidate_embeddings"], f32).reshape(M * K, D))
    starts = np.asarray(inputs["mention_starts"], np.int64)
    spans = np.asarray(inputs["span_lengths"], np.int64)
    ends = starts + spans

    j = np.stack([ends + 1, starts,
                  np.minimum(S - 1, ends + CTX),
                  np.maximum(0, starts - CTX)], axis=1)       # [M, 4]
    chunk_of = (np.maximum(j - 1, 0) // P).astype(np.int64)   # [M, 4]
    inv = np.stack([1.0 / (spans + 1).astype(f32),
                    1.0 / (j[:, 2] - j[:, 3]).astype(f32)], axis=1)

    consts = {
        "tri": np.triu(np.ones((P, P), f32)),
        "tri32": np.triu(np.ones((NCH, NCH), f32), k=1),
        "ident": np.eye(P, dtype=f32),
        "zrow": np.zeros((1, D), f32),
        "hmat": np.repeat(np.eye(H, dtype=f32), DH, axis=0).astype(bf16),
        "i8neg": (-np.eye(H, dtype=f32)).astype(bf16),
    }

    def fm(w, ic):
        """[Din, Dout] -> [P, Din//P, Dout] feature-major"""
        return np.ascontiguousarray(
            w.reshape(ic, P, w.shape[1]).transpose(1, 0, 2))

    wf = {n: np.asarray(inputs[n], f32) for n in
          ["relik_w1", "wq", "wk", "wv", "wo", "ffn_w1", "ffn_w2", "uni_w1"]}
    ln1_g = np.asarray(inputs["ln1_g"], f32)
    ln1_b = np.asarray(inputs["ln1_b"], f32)
    ln2_g = np.asarray(inputs["ln2_g"], f32)
    ln2_b = np.asarray(inputs["ln2_b"], f32)
    bo_v = np.asarray(inputs["bo"], f32)
    fb2_v = np.asarray(inputs["ffn_b2"], f32)
    # LN1 gain folded into ffn_w1 (x1 = g*xhat + b); bias into ffn_b1
    fw1_fold = ln1_g[:, None] * wf["ffn_w1"]
    w8 = {
        "wk8": fm(wf["wk"] * W8SCALE, FC).astype(fp8),
        "wv8": fm(wf["wv"] * W8SCALE, FC).astype(fp8),
        "wq8": fm(wf["wq"] * W8SCALE, FC).astype(fp8),
        "wo8": fm(wf["wo"] * W8SCALE, FC).astype(fp8),
        "u1b8": fm(wf["uni_w1"][D:] * W8SCALE, FC).astype(fp8),
        "fw18": fm(fw1_fold * W8SCALE, FC).astype(fp8),
        "fw28": fm(wf["ffn_w2"] * W8SCALE, HFC).astype(fp8),
        "w1bb": fm(wf["relik_w1"][D:], FC).astype(bf16),
    }
    dg64 = np.zeros((P, FC, P), f32)
    ar_p = np.arange(P)
    for c in range(FC):
        dg64[ar_p, c, ar_p] = W8SCALE * ln1_g[c * P:c * P + P]
    w8["dg64"] = dg64.astype(bf16)
    w8["sel16"] = np.ascontiguousarray(
        np.eye(16, dtype=f32)[:, :, None]
        * np.ones((1, 1, P), f32)).astype(bf16)
    sc_vals = np.array([bo_v.sum(), fb2_v.sum(), (ln2_g ** 2).sum(),
                        (ln2_g * ln2_b).sum(), (ln2_b ** 2).sum(),
                        (ln2_g ** 2 * fb2_v).sum(),
                        (ln2_g * ln2_b * fb2_v).sum(), 0.0], f32)
    w8["sconsts"] = np.ascontiguousarray(
        np.broadcast_to(sc_vals, (P, 8))).astype(f32)

    wnames = ["relik_w1", "relik_b1", "relik_w2",
              "wq", "bq", "wk", "bk", "wv", "bv", "bo",
              "ln1_b", "ffn_b2",
              "ln2_g", "ln2_b", "uni_w1", "uni_b1", "uni_w2"]
    weights = {n: np.ascontiguousarray(np.asarray(inputs[n], f32))
               for n in wnames}
    weights["ffn_b1"] = np.ascontiguousarray(
        np.asarray(inputs["ffn_b1"], f32) + ln1_b @ wf["ffn_w1"])
    weights["relik_b2"] = np.asarray(inputs["relik_b2"], f32).reshape(1, 1)
    weights["uni_b2"] = np.ascontiguousarray(
        np.asarray(inputs["uni_b2"], f32).reshape(1, D))

    in_maps = []
    for core in range(NCORES):
        sl = slice(core * M_LOC, (core + 1) * M_LOC)
        selt = np.zeros((NCH, 2, P), f32)
        jc = chunk_of[sl]                                     # [128, 4]
        ar = np.arange(P)
        for col, (tp, tm) in enumerate(((0, 1), (2, 3))):
            np.add.at(selt, (jc[:, tp], col, ar), 1.0)
            np.add.at(selt, (jc[:, tm], col, ar), -1.0)
        candT = np.ascontiguousarray(
            cand_full[core * PAIRS:(core + 1) * PAIRS].T
            .reshape(FC, P, PAIRS).transpose(1, 0, 2))        # [P, FC, PAIRS]
        im = {
            "txt": txt,
            "candT_bf": candT.astype(bf16),
            "candT_8": candT.astype(fp8),
            "idx": np.ascontiguousarray(j[sl].astype(np.int32)),
            "invl": np.ascontiguousarray(inv[sl].astype(f32)),
            "seltab": selt,
        }
        im.update(consts)
        im.update(weights)
        im.update(w8)
        in_maps.append(im)

    if "nc" not in _NC_CACHE:
        _NC_CACHE["nc"] = _build_nc()
    nc = _NC_CACHE["nc"]

    results = bass_utils.run_bass_kernel_spmd(
        nc, in_maps, core_ids=list(range(NCORES))).results

    out = np.zeros((3, M, K), f32)
    for core in range(NCORES):
        sl = slice(core * M_LOC, (core + 1) * M_LOC)
        out[:, sl, :] = results[core]["out"].reshape(3, M_LOC, K)
    return out


if __name__ == "__main__":
    nc = _build_nc()
    print("built ok")
